# revision 9
# baseline (speedup 1.0000x reference)
"""Trainium2 Bass kernel for nn_Encoder_6 (conv+GN+InterpLnr x3 -> biLSTM).

Self-contained: host-side prep (sharding, interp gather tables, weight
repacking) + Bass/Tile device kernel + output gather.

Data-parallel over 8 NeuronCores: 64 samples per core.

Device dataflow per core (all samples resident on-chip after one load):
  - activations live in [channel(partition), sample, time] layout
  - conv1d = 10-11 accumulating matmuls per sample-pair (taps x cin-chunks),
    f32r (tf32-like) dtype, PSUM [128, 2x192]
  - GroupNorm stats fused into PSUM evacuation (ACT copy+accum -> sums,
    DVE square+accum -> sumsq), group reduce + expand via tiny matmuls
  - normalize+ReLU = single ACT op (per-partition scale/bias)
  - InterpLnr = gpsimd ap_gather along time + 3 DVE ops (w1*g1 + w2*g2)
  - biLSTM: gate preacts via matmuls straight into PSUM blocks; recurrence
    reads gate slices from PSUM (ACT sigmoid/tanh, DVE cell update)
"""
import sys
from contextlib import ExitStack

sys.path.insert(0, "/opt/trn_rl_repo")

import numpy as np
import ml_dtypes

B = 512
N_CORES = 8
S = B // N_CORES          # samples per core
DIM_PIT = 257
C = 256                   # conv channels
T = 192                   # padded time
TH = 196                  # time with halo (2 each side)
GRP = 16                  # channels per group
DIM_NECK = 32
FREQ = 8
NT_OUT = 24               # output timesteps per direction
MIN_LEN_SEG = 19
MAX_NUM_SEG = 7
W64 = 64                  # 2*MAX_LEN_SEG
EPS = 1e-5
SG = 32                   # samples per stats group (2 groups per core)
NPAIR = 16                # sample pairs per stats group
LBLK = 4                  # LSTM timesteps per PSUM block
NBLK = T // LBLK

_cache = {}


# ---------------------------------------------------------------- host prep

def _interp_tables(scales_u, len_seg_raw, n):
    """Gather idx/w1/w2 per sample for one interp layer (numpy, exact)."""
    scales = scales_u.astype(np.float32) + np.float32(0.5)
    j = np.arange(W64, dtype=np.float32)
    idx_scaled = j[None, :] / scales[:, None]
    idx_fl = np.floor(idx_scaled)
    lam = idx_scaled - idx_fl
    len_seg = (len_seg_raw + MIN_LEN_SEG).astype(np.float32)[:, None]
    idx_mask = idx_fl < (len_seg - 1.0)
    ls = (len_seg_raw + MIN_LEN_SEG).reshape(n, MAX_NUM_SEG)
    offset = np.cumsum(ls, axis=-1)
    offset = np.pad(offset[:, :-1], ((0, 0), (1, 0))).reshape(-1, 1)
    idx_org = idx_fl + offset.astype(np.float32)
    mask = (idx_mask & (idx_org < (T - 1))).reshape(n, MAX_NUM_SEG * W64)
    idx_b = np.clip(idx_org.reshape(n, -1).astype(np.int32), 0, T - 2)
    lam_b = lam.reshape(n, -1)
    idx = np.zeros((n, T), np.int32)
    w1 = np.zeros((n, T), np.float32)
    w2 = np.zeros((n, T), np.float32)
    for b in range(n):
        js = np.nonzero(mask[b])[0][:T]
        k = len(js)
        idx[b, :k] = idx_b[b, js]
        w1[b, :k] = 1.0 - lam_b[b, js]
        w2[b, :k] = lam_b[b, js]
    return idx, w1, w2


def _wrap_idx(idx_pairs):
    """[n, NI] int -> ap_gather wrapped layout [n, 128, NI//16] int16."""
    n, NI = idx_pairs.shape
    wrapped = idx_pairs.reshape(n, NI // 16, 16).transpose(0, 2, 1)
    out = np.tile(wrapped[:, None, :, :], (1, 8, 1, 1)).reshape(n, 128, NI // 16)
    return np.ascontiguousarray(out.astype(np.int16))


def _prep_host(inputs):
    """Build per-core input dicts. Returns list of 8 dicts."""
    x = np.asarray(inputs["x"], np.float32)
    scales = np.asarray(inputs["scales"], np.float32)
    lsr = np.asarray(inputs["len_seg_raw"], np.int32)

    # conv weights as lhsT tiles [l, chunk, tap, half, cin128, cout128]
    wconv = np.zeros((3, 2, 5, 2, 128, 128), np.float32)
    for l in range(3):
        w = np.asarray(inputs[f"conv{l}_w"], np.float32)  # [256, cin, 5]
        for cc in range(2):
            for k in range(5):
                for h in range(2):
                    wconv[l, cc, k, h] = w[h * 128:(h + 1) * 128,
                                           cc * 128:(cc + 1) * 128, k].T
    wconv = np.ascontiguousarray(wconv)
    # conv0 channel 256 as [5, 256] lhsT (k=tap)
    w0 = np.asarray(inputs["conv0_w"], np.float32)
    wc0e = np.ascontiguousarray(w0[:, 256, :].T.astype(np.float16))  # [5, 256]

    conv_bias = [np.asarray(inputs[f"conv{l}_b"], np.float32) for l in range(3)]
    assert all(np.abs(b).max() == 0.0 for b in conv_bias), \
        "nonzero conv bias not implemented in device kernel"

    gamma_t = np.stack([np.asarray(inputs[f"gn{l}_g"], np.float32).reshape(2, 128)
                        for l in range(3)])          # [3, 2, 128]
    beta_t = np.stack([np.asarray(inputs[f"gn{l}_b"], np.float32).reshape(2, 128)
                       for l in range(3)])
    gamma_t = np.ascontiguousarray(gamma_t.transpose(2, 0, 1).reshape(128, 6))
    beta_t = np.ascontiguousarray(beta_t.transpose(2, 0, 1).reshape(128, 6))

    gind = np.zeros((128, 8), np.float32)
    for c in range(128):
        gind[c, c // 16] = 1.0
    gexp = np.ascontiguousarray(gind.T)               # [8, 128]

    # interp tables, all samples
    idx_all, w1_all, w2_all = [], [], []
    for l in range(3):
        idx, w1, w2 = _interp_tables(scales[l], lsr[l], B)
        idx_all.append(idx)
        w1_all.append(w1)
        w2_all.append(w2)

    # LSTM weights: gate reorder i,f,o,g; lhsT layouts
    def reord(a):  # [128, ...] gate-major
        i_, f_, g_, o_ = np.split(a, 4, axis=0)
        return np.concatenate([i_, f_, o_, g_], axis=0)

    # sample-partition LSTM layouts:
    #  wihT [cc, cin128, (d,gate)]  rhs of xw matmuls (bf16)
    #  whhT [d, h, gate]            rhs of recurrence matmuls (bf16)
    #  lbias4 [d, 4*128]            bias row tiled 4x for rank-1 (f32)
    wihT = np.zeros((2, 128, 256), np.float32)
    whhT = np.zeros((2, 32, 128), np.float32)
    lbias4 = np.zeros((2, 4, 128), np.float32)
    for d, nm in enumerate(["f", "b"]):
        wi = reord(np.asarray(inputs[f"w_ih_{nm}"], np.float32))   # [128, 256]
        wh = reord(np.asarray(inputs[f"w_hh_{nm}"], np.float32))   # [128, 32]
        bb = reord((np.asarray(inputs[f"b_ih_{nm}"], np.float32)
                    + np.asarray(inputs[f"b_hh_{nm}"], np.float32))[:, None])[:, 0]
        for cc in range(2):
            wihT[cc, :, d * 128:(d + 1) * 128] = wi[:, cc * 128:(cc + 1) * 128].T
        whhT[d] = wh.T
        lbias4[d, :, :] = bb[None, :]
    wihT = np.ascontiguousarray(wihT.astype(np.float16))
    whhT = np.ascontiguousarray(whhT.astype(np.float16))
    lbias4 = np.ascontiguousarray(lbias4.reshape(2, 512))

    in_maps = []
    for core in range(N_CORES):
        s0 = core * S
        xs = x[s0:s0 + S]                              # [S, 257, 192]
        xt = xs.transpose(1, 0, 2)                     # [257, S, 192]
        xa = np.zeros((128, S, TH), np.float32)
        xb = np.zeros((128, S, TH), np.float32)
        xa[:, :, 2:194] = xt[:128]
        xb[:, :, 2:194] = xt[128:256]
        xc = np.zeros((5, S, T), np.float32)
        x256 = xt[256]                                 # [S, 192]
        for k in range(5):
            sh = k - 2
            lo, hi = max(0, -sh), min(T, T - sh)
            xc[k, :, lo:hi] = x256[:, lo + sh:hi + sh]

        idxw = np.zeros((3, 128, S, 24), np.int16)
        wrep = np.zeros((3, S, 2, T), np.float32)
        for l in range(3):
            idx = idx_all[l][s0:s0 + S]
            pairs = np.concatenate([idx, idx + 1], axis=1)   # [S, 384]
            idxw[l] = _wrap_idx(pairs).transpose(1, 0, 2)    # [128, S, 24]
            wrep[l, :, 0] = w1_all[l][s0:s0 + S]
            wrep[l, :, 1] = w2_all[l][s0:s0 + S]
        # replicate interp weights across partitions: [3, S, 128, 2*T]
        wrepf = np.ascontiguousarray(
            np.broadcast_to(wrep[:, :, None, :, :], (3, S, 128, 2, T))
            .reshape(3, S, 128, 2 * T))

        in_maps.append({
            "xa": np.ascontiguousarray(xa),
            "xb": np.ascontiguousarray(xb),
            "xc": np.ascontiguousarray(xc.astype(np.float16)),
            "wconv": wconv,
            "wc0e": wc0e,
            "gamma_t": gamma_t,
            "beta_t": beta_t,
            "gind": gind,
            "gexp": gexp,
            "idxw": np.ascontiguousarray(idxw),
            "wrep": wrepf,
            "wihT": wihT,
            "whhT": whhT,
            "lbias4": lbias4,
        })
    return in_maps


# ------------------------------------------------------------- device build

def _build(probe_layer=-1):
    """Build the Bacc module. probe_layer >= 0 adds a probe output of XBUF
    after that layer's interp (for debugging)."""
    import concourse.bass as bass
    import concourse.tile as tile
    from concourse import bacc, mybir
    from concourse.masks import make_identity

    f32 = mybir.dt.float32
    f32r = mybir.dt.float32r
    bf16 = mybir.dt.bfloat16
    fp16 = mybir.dt.float16
    i16 = mybir.dt.int16
    AF = mybir.ActivationFunctionType
    OP = mybir.AluOpType

    nc = bacc.Bacc("TRN2", target_bir_lowering=False, debug=False,
                   enable_asserts=False, num_devices=N_CORES)

    # DRAM tensors
    d_xa = nc.dram_tensor("xa", [128, S, TH], f32r, kind="ExternalInput")
    d_xb = nc.dram_tensor("xb", [128, S, TH], f32r, kind="ExternalInput")
    d_xc = nc.dram_tensor("xc", [5, S, T], fp16, kind="ExternalInput")
    d_wconv = nc.dram_tensor("wconv", [3, 2, 5, 2, 128, 128], f32r,
                             kind="ExternalInput")
    d_wc0e = nc.dram_tensor("wc0e", [5, 256], fp16, kind="ExternalInput")
    d_gamma = nc.dram_tensor("gamma_t", [128, 6], f32, kind="ExternalInput")
    d_beta = nc.dram_tensor("beta_t", [128, 6], f32, kind="ExternalInput")
    d_gind = nc.dram_tensor("gind", [128, 8], f32, kind="ExternalInput")
    d_gexp = nc.dram_tensor("gexp", [8, 128], f32, kind="ExternalInput")
    d_idxw = nc.dram_tensor("idxw", [3, 128, S, 24], i16, kind="ExternalInput")
    d_wrep = nc.dram_tensor("wrep", [3, S, 128, 2 * T], f32,
                            kind="ExternalInput")
    d_wihT = nc.dram_tensor("wihT", [2, 128, 256], fp16, kind="ExternalInput")
    d_whhT = nc.dram_tensor("whhT", [2, 32, 128], fp16, kind="ExternalInput")
    d_lbias4 = nc.dram_tensor("lbias4", [2, 512], f32, kind="ExternalInput")
    d_out = nc.dram_tensor("out", [S, NT_OUT, 64], f32, kind="ExternalOutput")
    d_probe = None
    if probe_layer >= 0:
        d_probe = nc.dram_tensor("probe", [2, 128, S, TH], f32r,
                                 kind="ExternalOutput")

    es = ExitStack()
    with tile.TileContext(nc) as tc, es:
        consts = es.enter_context(tc.tile_pool(name="consts", bufs=1))
        xbufs = es.enter_context(tc.tile_pool(name="xbufs", bufs=1))

        # ---- constants
        t_xc = consts.tile([5, S, T], fp16)
        nc.sync.dma_start(out=t_xc[:], in_=d_xc[:, :, :])
        t_wc0e = consts.tile([5, 256], fp16)
        nc.sync.dma_start(out=t_wc0e[:], in_=d_wc0e[:, :])
        t_gamma = consts.tile([128, 6], f32)
        nc.sync.dma_start(out=t_gamma[:], in_=d_gamma[:, :])
        t_beta = consts.tile([128, 6], f32)
        nc.sync.dma_start(out=t_beta[:], in_=d_beta[:, :])
        t_gind = consts.tile([128, 8], f32)
        nc.sync.dma_start(out=t_gind[:], in_=d_gind[:, :])
        t_gexp = consts.tile([8, 128], f32)
        nc.sync.dma_start(out=t_gexp[:], in_=d_gexp[:, :])
        t_eps = consts.tile([8, 1], f32)
        nc.vector.memset(t_eps[:], EPS)
        t_ones = consts.tile([1, 512], f32)
        nc.vector.memset(t_ones[:], 1.0)
        t_wihT = consts.tile([128, 2, 256], fp16)
        nc.sync.dma_start(
            out=t_wihT[:],
            in_=bass.AP(tensor=d_wihT, offset=0,
                        ap=[[256, 128], [128 * 256, 2], [1, 256]]))
        t_whhT = consts.tile([32, 2, 128], fp16)
        nc.sync.dma_start(
            out=t_whhT[:],
            in_=bass.AP(tensor=d_whhT, offset=0,
                        ap=[[128, 32], [32 * 128, 2], [1, 128]]))
        t_lb4 = consts.tile([1, 2, 512], f32)
        nc.sync.dma_start(out=t_lb4[:], in_=d_lbias4[None, :, :])
        t_id64 = consts.tile([64, 64], f32)
        make_identity(nc, t_id64[:])

        # ---- input activations (xbuf reused as interp output every layer)
        t_xa = xbufs.tile([128, S, TH], f32r)
        t_xb = xbufs.tile([128, S, TH], f32r)
        nc.sync.dma_start(out=t_xa[:], in_=d_xa[:, :, :])
        nc.sync.dma_start(out=t_xb[:], in_=d_xb[:, :, :])
        xbuf = [t_xa, t_xb]

        def mm(out, lhsT, rhs, start, stop, dt=None, **kw):
            if dt is not None:
                lhsT = lhsT.bitcast(dt)
                rhs = rhs.bitcast(dt)
            nc.tensor.matmul(out=out, lhsT=lhsT, rhs=rhs, start=start,
                             stop=stop, **kw)

        # ================= conv + GN + interp layers =================
        with ExitStack() as ces:
            wpool = ces.enter_context(tc.tile_pool(name="wpool", bufs=1))
            hraw_p = ces.enter_context(tc.tile_pool(name="hraw", bufs=1))
            stats_p = ces.enter_context(tc.tile_pool(name="stats", bufs=2))
            small_p = ces.enter_context(tc.tile_pool(name="small", bufs=2))
            y_p = ces.enter_context(tc.tile_pool(name="ybuf", bufs=3))
            g_p = ces.enter_context(tc.tile_pool(name="gout", bufs=3))
            scr_p = ces.enter_context(tc.tile_pool(name="scr", bufs=3))
            w12_p = ces.enter_context(tc.tile_pool(name="w12", bufs=2))
            cpsum = ces.enter_context(
                tc.tile_pool(name="cpsum", bufs=4, space="PSUM"))
            spsum = ces.enter_context(
                tc.tile_pool(name="spsum", bufs=2, space="PSUM"))
            epsum = ces.enter_context(
                tc.tile_pool(name="epsum", bufs=2, space="PSUM"))

            for l in range(3):
                t_wc = wpool.tile([128, 20, 128], f32r, tag="wconv")
                nc.sync.dma_start(
                    out=t_wc[:],
                    in_=bass.AP(tensor=d_wconv, offset=l * 20 * 128 * 128,
                                ap=[[128, 128], [128 * 128, 20], [1, 128]]))
                t_idx = wpool.tile([128, S, 24], i16, tag="idxw")
                nc.sync.dma_start(out=t_idx[:], in_=d_idxw[l, :, :, :])

                for grp in range(2):
                    sums = [stats_p.tile([128, SG], f32, tag=f"sums{h}", name=f"sums{h}")
                            for h in range(2)]
                    qs = [stats_p.tile([128, SG], f32, tag=f"qs{h}", name=f"qs{h}")
                          for h in range(2)]
                    hraw = [hraw_p.tile([128, SG, T], fp16, tag=f"hraw{h}", name=f"hraw{h}")
                            for h in range(2)]

                    # ---- phase 1: conv + fused stats
                    for pp in range(NPAIR):
                        pr = grp * NPAIR + pp
                        for h in range(2):
                            ps = cpsum.tile([128, 2, T], f32, tag="cps")
                            ops = []
                            for cc in range(2):
                                for k in range(5):
                                    ops.append((
                                        t_wc[:, (cc * 5 + k) * 2 + h, :],
                                        xbuf[cc][:, 2 * pr:2 * pr + 2,
                                                 k:k + T], None))
                            if l == 0:
                                ops.append((
                                    t_wc0e[:, h * 128:(h + 1) * 128],
                                    t_xc[:, 2 * pr:2 * pr + 2, :], None))
                            for j, (lh, rh, dt) in enumerate(ops):
                                mm(ps[:], lh, rh, j == 0, j == len(ops) - 1,
                                   dt=dt)
                            for i in range(2):
                                sl = pp * 2 + i
                                nc.scalar.activation(
                                    out=hraw[h][:, sl, :], in_=ps[:, i, :],
                                    func=AF.Identity,
                                    accum_out=sums[h][:, sl:sl + 1])
                                scr = scr_p.tile([128, T], fp16, tag="sq")
                                nc.scalar.activation(
                                    out=scr[:], in_=ps[:, i, :],
                                    func=AF.Square,
                                    accum_out=qs[h][:, sl:sl + 1])

                    # ---- phase 2: group stats -> A, B per half
                    AB = []
                    for h in range(2):
                        g1 = spsum.tile([8, SG], f32, tag="gg")
                        mm(g1[:], t_gind[:], sums[h][:], True, True)
                        g2 = spsum.tile([8, SG], f32, tag="gg")
                        mm(g2[:], t_gind[:], qs[h][:], True, True)
                        mean = small_p.tile([8, SG], f32, tag="mean")
                        nc.vector.tensor_scalar_mul(mean[:], g1[:],
                                                    1.0 / (GRP * T))
                        msq = small_p.tile([8, SG], f32, tag="msq")
                        nc.vector.tensor_tensor(out=msq[:], in0=mean[:],
                                                in1=mean[:], op=OP.mult)
                        var = small_p.tile([8, SG], f32, tag="var")
                        nc.vector.scalar_tensor_tensor(
                            out=var[:], in0=g2[:], scalar=1.0 / (GRP * T),
                            in1=msq[:], op0=OP.mult, op1=OP.subtract)
                        sd = small_p.tile([8, SG], f32, tag="sd")
                        nc.scalar.activation(out=sd[:], in_=var[:],
                                             func=AF.Sqrt,
                                             bias=t_eps[:, :1], scale=1.0)
                        rstd = small_p.tile([8, SG], f32, tag="rstd")
                        nc.vector.reciprocal(rstd[:], sd[:])
                        rp = epsum.tile([128, SG], f32, tag="ee")
                        mm(rp[:], t_gexp[:], rstd[:], True, True)
                        mp = epsum.tile([128, SG], f32, tag="ee")
                        mm(mp[:], t_gexp[:], mean[:], True, True)
                        At = small_p.tile([128, SG], f32, tag="A")
                        nc.vector.tensor_scalar_mul(
                            At[:], rp[:],
                            t_gamma[:, l * 2 + h:l * 2 + h + 1])
                        tmp = small_p.tile([128, SG], f32, tag="tmp")
                        nc.vector.tensor_tensor(out=tmp[:], in0=mp[:],
                                                in1=At[:], op=OP.mult)
                        Bt = small_p.tile([128, SG], f32, tag="B")
                        nc.vector.tensor_scalar(
                            out=Bt[:], in0=tmp[:], scalar1=-1.0,
                            scalar2=t_beta[:, l * 2 + h:l * 2 + h + 1],
                            op0=OP.mult, op1=OP.add)
                        AB.append((At, Bt))

                    # ---- phase 3: normalize + relu + interp
                    for pp in range(NPAIR):
                        pr = grp * NPAIR + pp
                        w12 = w12_p.tile([128, 2, 2 * T], f32, tag="w12")
                        nc.sync.dma_start(
                            out=w12[:],
                            in_=d_wrep[l, 2 * pr:2 * pr + 2, :, :].rearrange(
                                "s p w -> p s w"))
                        for i in range(2):
                            sl = pp * 2 + i
                            sg_ = 2 * pr + i
                            for h in range(2):
                                At, Bt = AB[h]
                                yb = y_p.tile([128, T], f32, tag="yb")
                                nc.scalar.activation(
                                    out=yb[:], in_=hraw[h][:, sl, :],
                                    func=AF.Relu, bias=Bt[:, sl:sl + 1],
                                    scale=At[:, sl:sl + 1])
                                go = g_p.tile([128, 2, T], f32, tag="go")
                                nc.gpsimd.ap_gather(
                                    go[:], yb[:], t_idx[:, sg_, :],
                                    channels=128, num_elems=T, d=1,
                                    num_idxs=2 * T)
                                m1 = scr_p.tile([128, T], f32, tag="m1")
                                nc.vector.tensor_tensor(
                                    out=m1[:], in0=go[:, 0, :],
                                    in1=w12[:, i, 0:T], op=OP.mult)
                                m2 = scr_p.tile([128, T], f32, tag="m2")
                                nc.vector.tensor_tensor(
                                    out=m2[:], in0=go[:, 1, :],
                                    in1=w12[:, i, T:2 * T], op=OP.mult)
                                nc.vector.tensor_tensor(
                                    out=xbuf[h][:, sg_, 2:194], in0=m1[:],
                                    in1=m2[:], op=OP.add)

                if probe_layer == l:
                    for h in range(2):
                        nc.sync.dma_start(out=d_probe[h, :, :, :],
                                          in_=xbuf[h][:, :, :])

        # ======================= biLSTM =======================
        # sample-partition layout: [sample(64 part), gate-unit(col)].
        # h3 copied to bf16 for cheap xw matmuls.
        lsb = es.enter_context(tc.tile_pool(name="lstm_sbuf", bufs=1))
        h3bf = [lsb.tile([128, S, T], fp16, name=f"h3bf{cc}")
                for cc in range(2)]
        for cc in range(2):
            nc.scalar.activation(out=h3bf[cc][:], in_=xbuf[cc][:, :, 2:194],
                                 func=AF.Identity)
        t_SIG = lsb.tile([64, 256], f32)
        t_C = lsb.tile([64, 64], f32)
        t_U = lsb.tile([64, 64], f32)
        t_FC = lsb.tile([64, 64], f32)
        t_TC = lsb.tile([64, 64], f32)
        t_Hs = lsb.tile([64, 64], f32)
        t_HT = [lsb.tile([32, 64], fp16, name=f"ht{d}") for d in range(2)]
        t_OUT = lsb.tile([S, NT_OUT, 64], f32)
        nc.vector.memset(t_C[:], 0.0)
        nc.vector.memset(t_Hs[:], 0.0)
        for d in range(2):
            nc.vector.memset(t_HT[d][:], 0.0)

        with tc.tile_pool(name="lpsum", bufs=2, space="PSUM") as lpsum, \
             tc.tile_pool(name="tpsum", bufs=2, space="PSUM") as tpsum:

            def xw_block(blk, d):
                """gate preacts, 4 timesteps of dir d -> one PSUM bank."""
                ps = lpsum.tile([64, LBLK, 128], f32, tag=f"xw{d}",
                                name=f"xw{d}")
                for j in range(LBLK):
                    t = blk * LBLK + j if d == 0 else T - 1 - blk * LBLK - j
                    for cc in range(2):
                        mm(ps[:, j, :], h3bf[cc][:, :, t],
                           t_wihT[:, cc, d * 128:(d + 1) * 128],
                           j == 0 and cc == 0, False)
                mm(ps[:], t_ones[:, 0:64], t_lb4[:, d, :], False, True)
                return ps

            xwp = [[xw_block(0, d), None] for d in range(2)]

            for g in range(T):
                blk, j = g // LBLK, g % LBLK
                for d in range(2):
                    t = g if d == 0 else T - 1 - g
                    ps = xwp[d][blk % 2]
                    slc = ps[:, j, :]
                    dcol = d * 128
                    # gates += h_{t-1} @ whh
                    mm(slc, t_HT[d][:], t_whhT[:, d, :], False, True,
                       skip_group_check=True)
                    nc.scalar.activation(
                        out=t_SIG[:, dcol:dcol + 96], in_=slc[:, 0:96],
                        func=AF.Sigmoid)
                    nc.scalar.activation(
                        out=t_SIG[:, dcol + 96:dcol + 128],
                        in_=slc[:, 96:128], func=AF.Tanh)
                # strided-dir slices of t_SIG: [[128, 2], [1, 32]] at offset
                def sgs(off):
                    a = t_SIG[:]
                    return bass.AP(tensor=a.tensor, offset=a.offset + off,
                                   ap=[a.ap[0], [128, 2], [1, 32]])
                nc.vector.tensor_tensor(out=t_U[:], in0=sgs(0), in1=sgs(96),
                                        op=OP.mult)
                nc.vector.tensor_tensor(out=t_FC[:], in0=sgs(32),
                                        in1=t_C[:], op=OP.mult)
                nc.vector.tensor_tensor(out=t_C[:], in0=t_U[:], in1=t_FC[:],
                                        op=OP.add)
                nc.scalar.activation(out=t_TC[:], in_=t_C[:], func=AF.Tanh)
                nc.vector.tensor_tensor(out=t_Hs[:], in0=sgs(64),
                                        in1=t_TC[:], op=OP.mult)
                for d in range(2):
                    t = g if d == 0 else T - 1 - g
                    pht = tpsum.tile([32, 64], f32, tag="pht", name="pht")
                    nc.tensor.transpose(out=pht[:],
                                        in_=t_Hs[:, d * 32:(d + 1) * 32],
                                        identity=t_id64[:])
                    nc.scalar.activation(out=t_HT[d][:], in_=pht[:],
                                         func=AF.Identity)
                    if d == 0 and t % FREQ == FREQ - 1:
                        nc.vector.tensor_copy(
                            out=t_OUT[:, t // FREQ, 0:32],
                            in_=t_Hs[:, 0:32])
                    if d == 1 and t % FREQ == 0:
                        nc.vector.tensor_copy(
                            out=t_OUT[:, t // FREQ, 32:64],
                            in_=t_Hs[:, 32:64])
                if j == 1 and blk + 1 < NBLK:
                    for d in range(2):
                        xwp[d][(blk + 1) % 2] = xw_block(blk + 1, d)

        nc.sync.dma_start(out=d_out[:, :, :], in_=t_OUT[:])

    nc.compile()
    return nc


def _get_nc(probe_layer=-1):
    key = ("nc", probe_layer)
    if key not in _cache:
        _cache[key] = _build(probe_layer)
    return _cache[key]


def run_on_cores(inputs, probe_layer=-1, trace=False):
    """Build (cached), run on 8 cores; returns (results, BassKernelResults)."""
    from concourse.bass_utils import run_bass_kernel_spmd

    nc = _get_nc(probe_layer)
    in_maps = _prep_host(inputs)
    last_exc = None
    for _ in range(3):
        try:
            res = run_bass_kernel_spmd(nc, in_maps,
                                       core_ids=list(range(N_CORES)),
                                       trace=trace)
            return res
        except Exception as e:  # transient NRT errors happen; retry
            last_exc = e
    raise last_exc


def assemble_output(res):
    out = np.zeros((B, NT_OUT, 64), np.float32)
    for core in range(N_CORES):
        s0 = core * S
        out[s0:s0 + S] = res.results[core]["out"]
    return out


def kernel(**inputs):
    res = run_on_cores(inputs)
    return assemble_output(res)


# revision 11
# speedup vs baseline: 3.1401x; 3.1401x over previous
"""Trainium2 Bass kernel for nn_Encoder_6 (conv+GN+InterpLnr x3 -> biLSTM).

Self-contained: host-side prep (sharding, interp gather tables, weight
repacking) + Bass/Tile device kernel + output gather.

Data-parallel over 8 NeuronCores: 64 samples per core.

Device dataflow per core (all samples resident on-chip after one load):
  - activations live in [channel(partition), sample, time] layout
  - conv1d = 10-11 accumulating matmuls per sample-pair (taps x cin-chunks),
    f32r (tf32-like) dtype, PSUM [128, 2x192]
  - GroupNorm stats fused into PSUM evacuation (ACT copy+accum -> sums,
    DVE square+accum -> sumsq), group reduce + expand via tiny matmuls
  - normalize+ReLU = single ACT op (per-partition scale/bias)
  - InterpLnr = gpsimd ap_gather along time + 3 DVE ops (w1*g1 + w2*g2)
  - biLSTM: gate preacts via matmuls straight into PSUM blocks; recurrence
    reads gate slices from PSUM (ACT sigmoid/tanh, DVE cell update)
"""
import sys
from contextlib import ExitStack

sys.path.insert(0, "/opt/trn_rl_repo")

import numpy as np
import ml_dtypes

B = 512
N_CORES = 8
S = B // N_CORES          # samples per core
DIM_PIT = 257
C = 256                   # conv channels
T = 192                   # padded time
TH = 196                  # time with halo (2 each side)
GRP = 16                  # channels per group
DIM_NECK = 32
FREQ = 8
NT_OUT = 24               # output timesteps per direction
MIN_LEN_SEG = 19
MAX_NUM_SEG = 7
W64 = 64                  # 2*MAX_LEN_SEG
EPS = 1e-5
SG = 32                   # samples per stats group (2 groups per core)
NPAIR = 16                # sample pairs per stats group
LBLK = 4                  # LSTM timesteps per PSUM block
NBLK = T // LBLK

_cache = {}


# ---------------------------------------------------------------- host prep

def _interp_tables(scales_u, len_seg_raw, n):
    """Gather idx/w1/w2 per sample for one interp layer (numpy, exact)."""
    scales = scales_u.astype(np.float32) + np.float32(0.5)
    j = np.arange(W64, dtype=np.float32)
    idx_scaled = j[None, :] / scales[:, None]
    idx_fl = np.floor(idx_scaled)
    lam = idx_scaled - idx_fl
    len_seg = (len_seg_raw + MIN_LEN_SEG).astype(np.float32)[:, None]
    idx_mask = idx_fl < (len_seg - 1.0)
    ls = (len_seg_raw + MIN_LEN_SEG).reshape(n, MAX_NUM_SEG)
    offset = np.cumsum(ls, axis=-1)
    offset = np.pad(offset[:, :-1], ((0, 0), (1, 0))).reshape(-1, 1)
    idx_org = idx_fl + offset.astype(np.float32)
    mask = (idx_mask & (idx_org < (T - 1))).reshape(n, MAX_NUM_SEG * W64)
    idx_b = np.clip(idx_org.reshape(n, -1).astype(np.int32), 0, T - 2)
    lam_b = lam.reshape(n, -1)
    idx = np.zeros((n, T), np.int32)
    w1 = np.zeros((n, T), np.float32)
    w2 = np.zeros((n, T), np.float32)
    for b in range(n):
        js = np.nonzero(mask[b])[0][:T]
        k = len(js)
        idx[b, :k] = idx_b[b, js]
        w1[b, :k] = 1.0 - lam_b[b, js]
        w2[b, :k] = lam_b[b, js]
    return idx, w1, w2


def _wrap_idx(idx_pairs):
    """[n, NI] int -> ap_gather wrapped layout [n, 128, NI//16] int16."""
    n, NI = idx_pairs.shape
    wrapped = idx_pairs.reshape(n, NI // 16, 16).transpose(0, 2, 1)
    out = np.tile(wrapped[:, None, :, :], (1, 8, 1, 1)).reshape(n, 128, NI // 16)
    return np.ascontiguousarray(out.astype(np.int16))


def _prep_host(inputs):
    """Build per-core input dicts. Returns list of 8 dicts."""
    x = np.asarray(inputs["x"], np.float32)
    scales = np.asarray(inputs["scales"], np.float32)
    lsr = np.asarray(inputs["len_seg_raw"], np.int32)

    # conv weights as lhsT tiles [l, chunk, tap, half, cin128, cout128]
    wconv = np.zeros((3, 2, 5, 2, 128, 128), np.float32)
    for l in range(3):
        w = np.asarray(inputs[f"conv{l}_w"], np.float32)  # [256, cin, 5]
        for cc in range(2):
            for k in range(5):
                for h in range(2):
                    wconv[l, cc, k, h] = w[h * 128:(h + 1) * 128,
                                           cc * 128:(cc + 1) * 128, k].T
    wconv = np.ascontiguousarray(wconv)
    # conv0 channel 256 as [5, 256] lhsT (k=tap)
    w0 = np.asarray(inputs["conv0_w"], np.float32)
    wc0e = np.ascontiguousarray(w0[:, 256, :].T.astype(np.float16))  # [5, 256]

    conv_bias = [np.asarray(inputs[f"conv{l}_b"], np.float32) for l in range(3)]
    assert all(np.abs(b).max() == 0.0 for b in conv_bias), \
        "nonzero conv bias not implemented in device kernel"

    gamma_t = np.stack([np.asarray(inputs[f"gn{l}_g"], np.float32).reshape(2, 128)
                        for l in range(3)])          # [3, 2, 128]
    beta_t = np.stack([np.asarray(inputs[f"gn{l}_b"], np.float32).reshape(2, 128)
                       for l in range(3)])
    gamma_t = np.ascontiguousarray(gamma_t.transpose(2, 0, 1).reshape(128, 6))
    beta_t = np.ascontiguousarray(beta_t.transpose(2, 0, 1).reshape(128, 6))

    gind = np.zeros((128, 8), np.float32)
    for c in range(128):
        gind[c, c // 16] = 1.0
    gexp = np.ascontiguousarray(gind.T)               # [8, 128]

    # interp tables, all samples
    idx_all, w1_all, w2_all = [], [], []
    for l in range(3):
        idx, w1, w2 = _interp_tables(scales[l], lsr[l], B)
        idx_all.append(idx)
        w1_all.append(w1)
        w2_all.append(w2)

    # LSTM weights: gate reorder i,f,o,g; lhsT layouts
    def reord(a):  # [128, ...] gate-major
        i_, f_, g_, o_ = np.split(a, 4, axis=0)
        return np.concatenate([i_, f_, o_, g_], axis=0)

    # sample-partition LSTM layouts:
    #  wihT [cc, cin128, (d,gate)]  rhs of xw matmuls (bf16)
    #  whhT [d, h, gate]            rhs of recurrence matmuls (bf16)
    #  lbias4 [d, 4*128]            bias row tiled 4x for rank-1 (f32)
    wihT = np.zeros((2, 128, 256), np.float32)
    whhT = np.zeros((2, 32, 128), np.float32)
    lbias4 = np.zeros((2, 4, 128), np.float32)
    for d, nm in enumerate(["f", "b"]):
        wi = reord(np.asarray(inputs[f"w_ih_{nm}"], np.float32))   # [128, 256]
        wh = reord(np.asarray(inputs[f"w_hh_{nm}"], np.float32))   # [128, 32]
        bb = reord((np.asarray(inputs[f"b_ih_{nm}"], np.float32)
                    + np.asarray(inputs[f"b_hh_{nm}"], np.float32))[:, None])[:, 0]
        for cc in range(2):
            wihT[cc, :, d * 128:(d + 1) * 128] = wi[:, cc * 128:(cc + 1) * 128].T
        whhT[d] = wh.T
        lbias4[d, :, :] = bb[None, :]
    wihT = np.ascontiguousarray(wihT.astype(np.float16))
    whhT = np.ascontiguousarray(whhT.astype(np.float16))
    lbias4 = np.ascontiguousarray(lbias4.reshape(2, 512))

    in_maps = []
    for core in range(N_CORES):
        s0 = core * S
        xs = x[s0:s0 + S]                              # [S, 257, 192]
        xt = xs.transpose(1, 0, 2)                     # [257, S, 192]
        xa = np.zeros((128, S, TH), np.float32)
        xb = np.zeros((128, S, TH), np.float32)
        xa[:, :, 2:194] = xt[:128]
        xb[:, :, 2:194] = xt[128:256]
        xc = np.zeros((5, S, T), np.float32)
        x256 = xt[256]                                 # [S, 192]
        for k in range(5):
            sh = k - 2
            lo, hi = max(0, -sh), min(T, T - sh)
            xc[k, :, lo:hi] = x256[:, lo + sh:hi + sh]

        # banded interp matrices S[t_in, t_out] per (layer, sample), fp16
        wS = np.zeros((3, S, T, T), np.float16)
        bi = np.arange(S)[:, None]
        pj = np.arange(T)[None, :]
        for l in range(3):
            idx = idx_all[l][s0:s0 + S]
            Sm = np.zeros((S, T, T), np.float32)
            Sm[bi, idx, pj] = w1_all[l][s0:s0 + S]
            Sm[bi, idx + 1, pj] += w2_all[l][s0:s0 + S]
            wS[l] = Sm.astype(np.float16)

        in_maps.append({
            "xa": np.ascontiguousarray(xa),
            "xb": np.ascontiguousarray(xb),
            "xc": np.ascontiguousarray(xc.astype(np.float16)),
            "wconv": wconv,
            "wc0e": wc0e,
            "gamma_t": gamma_t,
            "beta_t": beta_t,
            "gind": gind,
            "gexp": gexp,
            "wS": np.ascontiguousarray(wS),
            "id128": np.eye(128, dtype=np.float16),
            "wihT": wihT,
            "whhT": whhT,
            "lbias4": lbias4,
        })
    return in_maps


# ------------------------------------------------------------- device build

def _build(probe_layer=-1):
    """Build the Bacc module. probe_layer >= 0 adds a probe output of XBUF
    after that layer's interp (for debugging)."""
    import concourse.bass as bass
    import concourse.tile as tile
    from concourse import bacc, mybir
    from concourse.masks import make_identity

    f32 = mybir.dt.float32
    f32r = mybir.dt.float32r
    bf16 = mybir.dt.bfloat16
    fp16 = mybir.dt.float16
    i16 = mybir.dt.int16
    AF = mybir.ActivationFunctionType
    OP = mybir.AluOpType

    nc = bacc.Bacc("TRN2", target_bir_lowering=False, debug=False,
                   enable_asserts=False, num_devices=N_CORES)

    # DRAM tensors
    d_xa = nc.dram_tensor("xa", [128, S, TH], f32r, kind="ExternalInput")
    d_xb = nc.dram_tensor("xb", [128, S, TH], f32r, kind="ExternalInput")
    d_xc = nc.dram_tensor("xc", [5, S, T], fp16, kind="ExternalInput")
    d_wconv = nc.dram_tensor("wconv", [3, 2, 5, 2, 128, 128], f32r,
                             kind="ExternalInput")
    d_wc0e = nc.dram_tensor("wc0e", [5, 256], fp16, kind="ExternalInput")
    d_gamma = nc.dram_tensor("gamma_t", [128, 6], f32, kind="ExternalInput")
    d_beta = nc.dram_tensor("beta_t", [128, 6], f32, kind="ExternalInput")
    d_gind = nc.dram_tensor("gind", [128, 8], f32, kind="ExternalInput")
    d_gexp = nc.dram_tensor("gexp", [8, 128], f32, kind="ExternalInput")
    d_wS = nc.dram_tensor("wS", [3, S, T, T], fp16, kind="ExternalInput")
    d_id128 = nc.dram_tensor("id128", [128, 128], fp16, kind="ExternalInput")
    d_wihT = nc.dram_tensor("wihT", [2, 128, 256], fp16, kind="ExternalInput")
    d_whhT = nc.dram_tensor("whhT", [2, 32, 128], fp16, kind="ExternalInput")
    d_lbias4 = nc.dram_tensor("lbias4", [2, 512], f32, kind="ExternalInput")
    d_out = nc.dram_tensor("out", [S, NT_OUT, 64], f32, kind="ExternalOutput")
    d_probe = None
    if probe_layer >= 0:
        d_probe = nc.dram_tensor("probe", [2, 128, S, TH], f32r,
                                 kind="ExternalOutput")

    es = ExitStack()
    with tile.TileContext(nc) as tc, es:
        consts = es.enter_context(tc.tile_pool(name="consts", bufs=1))
        xbufs = es.enter_context(tc.tile_pool(name="xbufs", bufs=1))

        # ---- constants
        t_xc = consts.tile([5, S, T], fp16)
        nc.sync.dma_start(out=t_xc[:], in_=d_xc[:, :, :])
        t_wc0e = consts.tile([5, 256], fp16)
        nc.sync.dma_start(out=t_wc0e[:], in_=d_wc0e[:, :])
        t_gamma = consts.tile([128, 6], f32)
        nc.sync.dma_start(out=t_gamma[:], in_=d_gamma[:, :])
        t_beta = consts.tile([128, 6], f32)
        nc.sync.dma_start(out=t_beta[:], in_=d_beta[:, :])
        t_gind = consts.tile([128, 8], f32)
        nc.sync.dma_start(out=t_gind[:], in_=d_gind[:, :])
        t_gexp = consts.tile([8, 128], f32)
        nc.sync.dma_start(out=t_gexp[:], in_=d_gexp[:, :])
        t_eps = consts.tile([8, 1], f32)
        nc.vector.memset(t_eps[:], EPS)
        t_ones = consts.tile([1, 512], f32)
        nc.vector.memset(t_ones[:], 1.0)
        t_wihT = consts.tile([128, 2, 256], fp16)
        nc.sync.dma_start(
            out=t_wihT[:],
            in_=bass.AP(tensor=d_wihT, offset=0,
                        ap=[[256, 128], [128 * 256, 2], [1, 256]]))
        t_whhT = consts.tile([32, 2, 128], fp16)
        nc.sync.dma_start(
            out=t_whhT[:],
            in_=bass.AP(tensor=d_whhT, offset=0,
                        ap=[[128, 32], [32 * 128, 2], [1, 128]]))
        t_lb4 = consts.tile([1, 2, 512], f32)
        nc.sync.dma_start(out=t_lb4[:], in_=d_lbias4[None, :, :])
        t_id64 = consts.tile([64, 64], f32)
        make_identity(nc, t_id64[:])
        t_id128 = consts.tile([128, 128], fp16)
        nc.sync.dma_start(out=t_id128[:], in_=d_id128[:, :])

        # ---- input activations (xbuf reused as interp output every layer)
        t_xa = xbufs.tile([128, S, TH], f32r)
        t_xb = xbufs.tile([128, S, TH], f32r)
        nc.sync.dma_start(out=t_xa[:], in_=d_xa[:, :, :])
        nc.sync.dma_start(out=t_xb[:], in_=d_xb[:, :, :])
        xbuf = [t_xa, t_xb]

        def mm(out, lhsT, rhs, start, stop, dt=None, **kw):
            if dt is not None:
                lhsT = lhsT.bitcast(dt)
                rhs = rhs.bitcast(dt)
            nc.tensor.matmul(out=out, lhsT=lhsT, rhs=rhs, start=start,
                             stop=stop, **kw)

        # ================= conv + GN + interp layers =================
        with ExitStack() as ces:
            wpool = ces.enter_context(tc.tile_pool(name="wpool", bufs=1))
            hraw_p = ces.enter_context(tc.tile_pool(name="hraw", bufs=1))
            stats_p = ces.enter_context(tc.tile_pool(name="stats", bufs=2))
            small_p = ces.enter_context(tc.tile_pool(name="small", bufs=2))
            y_p = ces.enter_context(tc.tile_pool(name="ybuf", bufs=3))
            scr_p = ces.enter_context(tc.tile_pool(name="scr", bufs=3))
            sm_p = ces.enter_context(tc.tile_pool(name="smat", bufs=2))
            yt_p = ces.enter_context(tc.tile_pool(name="ytp", bufs=3))
            cpsum = ces.enter_context(
                tc.tile_pool(name="cpsum", bufs=2, space="PSUM"))
            stps = ces.enter_context(
                tc.tile_pool(name="stps", bufs=2, space="PSUM"))
            tpsum = ces.enter_context(
                tc.tile_pool(name="tpsum", bufs=2, space="PSUM"))
            sops = ces.enter_context(
                tc.tile_pool(name="sops", bufs=2, space="PSUM"))

            for l in range(3):
                t_wc = wpool.tile([128, 20, 128], f32r, tag="wconv")
                nc.sync.dma_start(
                    out=t_wc[:],
                    in_=bass.AP(tensor=d_wconv, offset=l * 20 * 128 * 128,
                                ap=[[128, 128], [128 * 128, 20], [1, 128]]))

                for grp in range(2):
                    sums = [stats_p.tile([128, SG], f32, tag=f"sums{h}", name=f"sums{h}")
                            for h in range(2)]
                    qs = [stats_p.tile([128, SG], f32, tag=f"qs{h}", name=f"qs{h}")
                          for h in range(2)]
                    hraw = [hraw_p.tile([128, SG, T], fp16, tag=f"hraw{h}", name=f"hraw{h}")
                            for h in range(2)]

                    # ---- phase 1: conv + fused stats
                    for pp in range(NPAIR):
                        pr = grp * NPAIR + pp
                        for h in range(2):
                            ps = cpsum.tile([128, 2, T], f32, tag="cps")
                            ops = []
                            for cc in range(2):
                                for k in range(5):
                                    ops.append((
                                        t_wc[:, (cc * 5 + k) * 2 + h, :],
                                        xbuf[cc][:, 2 * pr:2 * pr + 2,
                                                 k:k + T], None))
                            if l == 0:
                                ops.append((
                                    t_wc0e[:, h * 128:(h + 1) * 128],
                                    t_xc[:, 2 * pr:2 * pr + 2, :], None))
                            for j, (lh, rh, dt) in enumerate(ops):
                                mm(ps[:], lh, rh, j == 0, j == len(ops) - 1,
                                   dt=dt)
                            for i in range(2):
                                sl = pp * 2 + i
                                nc.scalar.activation(
                                    out=hraw[h][:, sl, :], in_=ps[:, i, :],
                                    func=AF.Identity,
                                    accum_out=sums[h][:, sl:sl + 1])
                                scr = scr_p.tile([128, T], fp16, tag="sq")
                                nc.vector.scalar_tensor_tensor(
                                    out=scr[:], in0=ps[:, i, :], scalar=1.0,
                                    in1=hraw[h][:, sl, :], op0=OP.mult,
                                    op1=OP.mult,
                                    accum_out=qs[h][:, sl:sl + 1])

                    # ---- phase 2: group stats -> A, B per half
                    AB = []
                    for h in range(2):
                        g1 = stps.tile([8, SG], f32, tag="gg")
                        mm(g1[:], t_gind[:], sums[h][:], True, True)
                        g2 = stps.tile([8, SG], f32, tag="gg")
                        mm(g2[:], t_gind[:], qs[h][:], True, True)
                        mean = small_p.tile([8, SG], f32, tag="mean")
                        nc.vector.tensor_scalar_mul(mean[:], g1[:],
                                                    1.0 / (GRP * T))
                        msq = small_p.tile([8, SG], f32, tag="msq")
                        nc.vector.tensor_tensor(out=msq[:], in0=mean[:],
                                                in1=mean[:], op=OP.mult)
                        var = small_p.tile([8, SG], f32, tag="var")
                        nc.vector.scalar_tensor_tensor(
                            out=var[:], in0=g2[:], scalar=1.0 / (GRP * T),
                            in1=msq[:], op0=OP.mult, op1=OP.subtract)
                        sd = small_p.tile([8, SG], f32, tag="sd")
                        nc.scalar.activation(out=sd[:], in_=var[:],
                                             func=AF.Sqrt,
                                             bias=t_eps[:, :1], scale=1.0)
                        rstd = small_p.tile([8, SG], f32, tag="rstd")
                        nc.vector.reciprocal(rstd[:], sd[:])
                        rp = stps.tile([128, SG], f32, tag="gg")
                        mm(rp[:], t_gexp[:], rstd[:], True, True)
                        mp = stps.tile([128, SG], f32, tag="gg")
                        mm(mp[:], t_gexp[:], mean[:], True, True)
                        At = small_p.tile([128, SG], f32, tag="A")
                        nc.vector.tensor_scalar_mul(
                            At[:], rp[:],
                            t_gamma[:, l * 2 + h:l * 2 + h + 1])
                        tmp = small_p.tile([128, SG], f32, tag="tmp")
                        nc.vector.tensor_tensor(out=tmp[:], in0=mp[:],
                                                in1=At[:], op=OP.mult)
                        Bt = small_p.tile([128, SG], f32, tag="B")
                        nc.vector.tensor_scalar(
                            out=Bt[:], in0=tmp[:], scalar1=-1.0,
                            scalar2=t_beta[:, l * 2 + h:l * 2 + h + 1],
                            op0=OP.mult, op1=OP.add)
                        AB.append((At, Bt))

                    # ---- phase 3: normalize+relu, transpose, interp matmul
                    for pp in range(NPAIR):
                        pr = grp * NPAIR + pp
                        s128 = sm_p.tile([128, 2, T], fp16, tag="s128")
                        nc.gpsimd.dma_start(
                            out=s128[:],
                            in_=d_wS[l, 2 * pr:2 * pr + 2, 0:128, :].rearrange(
                                "s t w -> t s w"))
                        s64 = sm_p.tile([64, 2, T], fp16, tag="s64")
                        nc.gpsimd.dma_start(
                            out=s64[:],
                            in_=d_wS[l, 2 * pr:2 * pr + 2, 128:192, :].rearrange(
                                "s t w -> t s w"))
                        for i in range(2):
                            sl = pp * 2 + i
                            sg_ = 2 * pr + i
                            yt128 = yt_p.tile([128, 256], fp16, tag="yt128")
                            yt64 = yt_p.tile([64, 256], fp16, tag="yt64")
                            for h in range(2):
                                At, Bt = AB[h]
                                yb = y_p.tile([128, T], fp16, tag="yb")
                                nc.scalar.activation(
                                    out=yb[:], in_=hraw[h][:, sl, :],
                                    func=AF.Relu, bias=Bt[:, sl:sl + 1],
                                    scale=At[:, sl:sl + 1])
                                ptp = tpsum.tile([128, 2, 128], fp16,
                                                 tag="tp", name="ptp")
                                nc.tensor.transpose(
                                    out=ptp[:, 0, :], in_=yb[:, 0:128],
                                    identity=t_id128[:])
                                nc.tensor.transpose(
                                    out=ptp[0:64, 1, :], in_=yb[:, 128:192],
                                    identity=t_id128[:])
                                nc.scalar.activation(
                                    out=yt128[:, h * 128:(h + 1) * 128],
                                    in_=ptp[:, 0, :], func=AF.Identity)
                                nc.vector.tensor_copy(
                                    out=yt64[:, h * 128:(h + 1) * 128],
                                    in_=ptp[0:64, 1, :])
                            sout = sops.tile([128, 2, T], f32, tag="so",
                                             name="sout")
                            for ch in range(2):
                                mm(sout[:, ch, :],
                                   yt128[:, ch * 128:(ch + 1) * 128],
                                   s128[:, i, :], True, False)
                                mm(sout[:, ch, :],
                                   yt64[:, ch * 128:(ch + 1) * 128],
                                   s64[:, i, :], False, True)
                            nc.vector.tensor_copy(
                                out=xbuf[0][:, sg_, 2:194], in_=sout[:, 0, :])
                            nc.vector.tensor_copy(
                                out=xbuf[1][:, sg_, 2:194], in_=sout[:, 1, :])

                if probe_layer == l:
                    for h in range(2):
                        nc.sync.dma_start(out=d_probe[h, :, :, :],
                                          in_=xbuf[h][:, :, :])

        # ======================= biLSTM =======================
        # sample-partition layout: [sample(64 part), gate-unit(col)].
        # h3 copied to bf16 for cheap xw matmuls.
        lsb = es.enter_context(tc.tile_pool(name="lstm_sbuf", bufs=1))
        h3bf = [lsb.tile([128, S, T], fp16, name=f"h3bf{cc}")
                for cc in range(2)]
        for cc in range(2):
            nc.scalar.activation(out=h3bf[cc][:], in_=xbuf[cc][:, :, 2:194],
                                 func=AF.Identity)
        t_SIG = lsb.tile([64, 256], f32)
        t_C = lsb.tile([64, 64], f32)
        t_U = lsb.tile([64, 64], f32)
        t_FC = lsb.tile([64, 64], f32)
        t_TC = lsb.tile([64, 64], f32)
        t_Hs = lsb.tile([64, 64], f32)
        t_HT = [lsb.tile([32, 64], fp16, name=f"ht{d}") for d in range(2)]
        t_OUT = lsb.tile([S, NT_OUT, 64], f32)
        nc.vector.memset(t_C[:], 0.0)
        nc.vector.memset(t_Hs[:], 0.0)
        for d in range(2):
            nc.vector.memset(t_HT[d][:], 0.0)

        with tc.tile_pool(name="lpsum", bufs=2, space="PSUM") as lpsum, \
             tc.tile_pool(name="tpsum", bufs=2, space="PSUM") as tpsum:

            def xw_block(blk, d):
                """gate preacts, 4 timesteps of dir d -> one PSUM bank."""
                ps = lpsum.tile([64, LBLK, 128], f32, tag=f"xw{d}",
                                name=f"xw{d}")
                for j in range(LBLK):
                    t = blk * LBLK + j if d == 0 else T - 1 - blk * LBLK - j
                    for cc in range(2):
                        mm(ps[:, j, :], h3bf[cc][:, :, t],
                           t_wihT[:, cc, d * 128:(d + 1) * 128],
                           j == 0 and cc == 0, False)
                mm(ps[:], t_ones[:, 0:64], t_lb4[:, d, :], False, True)
                return ps

            xwp = [[xw_block(0, d), None] for d in range(2)]

            for g in range(T):
                blk, j = g // LBLK, g % LBLK
                for d in range(2):
                    t = g if d == 0 else T - 1 - g
                    ps = xwp[d][blk % 2]
                    slc = ps[:, j, :]
                    dcol = d * 128
                    # gates += h_{t-1} @ whh
                    mm(slc, t_HT[d][:], t_whhT[:, d, :], False, True,
                       skip_group_check=True)
                    nc.scalar.activation(
                        out=t_SIG[:, dcol:dcol + 96], in_=slc[:, 0:96],
                        func=AF.Sigmoid)
                    nc.scalar.activation(
                        out=t_SIG[:, dcol + 96:dcol + 128],
                        in_=slc[:, 96:128], func=AF.Tanh)
                # strided-dir slices of t_SIG: [[128, 2], [1, 32]] at offset
                def sgs(off):
                    a = t_SIG[:]
                    return bass.AP(tensor=a.tensor, offset=a.offset + off,
                                   ap=[a.ap[0], [128, 2], [1, 32]])
                nc.vector.tensor_tensor(out=t_U[:], in0=sgs(0), in1=sgs(96),
                                        op=OP.mult)
                nc.vector.tensor_tensor(out=t_FC[:], in0=sgs(32),
                                        in1=t_C[:], op=OP.mult)
                nc.vector.tensor_tensor(out=t_C[:], in0=t_U[:], in1=t_FC[:],
                                        op=OP.add)
                nc.scalar.activation(out=t_TC[:], in_=t_C[:], func=AF.Tanh)
                nc.vector.tensor_tensor(out=t_Hs[:], in0=sgs(64),
                                        in1=t_TC[:], op=OP.mult)
                for d in range(2):
                    t = g if d == 0 else T - 1 - g
                    pht = tpsum.tile([32, 64], f32, tag="pht", name="pht")
                    nc.tensor.transpose(out=pht[:],
                                        in_=t_Hs[:, d * 32:(d + 1) * 32],
                                        identity=t_id64[:])
                    nc.scalar.activation(out=t_HT[d][:], in_=pht[:],
                                         func=AF.Identity)
                    if d == 0 and t % FREQ == FREQ - 1:
                        nc.vector.tensor_copy(
                            out=t_OUT[:, t // FREQ, 0:32],
                            in_=t_Hs[:, 0:32])
                    if d == 1 and t % FREQ == 0:
                        nc.vector.tensor_copy(
                            out=t_OUT[:, t // FREQ, 32:64],
                            in_=t_Hs[:, 32:64])
                if j == 1 and blk + 1 < NBLK:
                    for d in range(2):
                        xwp[d][(blk + 1) % 2] = xw_block(blk + 1, d)

        nc.sync.dma_start(out=d_out[:, :, :], in_=t_OUT[:])

    nc.compile()
    return nc


def _get_nc(probe_layer=-1):
    key = ("nc", probe_layer)
    if key not in _cache:
        _cache[key] = _build(probe_layer)
    return _cache[key]


def run_on_cores(inputs, probe_layer=-1, trace=False):
    """Build (cached), run on 8 cores; returns (results, BassKernelResults)."""
    from concourse.bass_utils import run_bass_kernel_spmd

    nc = _get_nc(probe_layer)
    in_maps = _prep_host(inputs)
    last_exc = None
    for _ in range(3):
        try:
            res = run_bass_kernel_spmd(nc, in_maps,
                                       core_ids=list(range(N_CORES)),
                                       trace=trace)
            return res
        except Exception as e:  # transient NRT errors happen; retry
            last_exc = e
    raise last_exc


def assemble_output(res):
    out = np.zeros((B, NT_OUT, 64), np.float32)
    for core in range(N_CORES):
        s0 = core * S
        out[s0:s0 + S] = res.results[core]["out"]
    return out


def kernel(**inputs):
    res = run_on_cores(inputs)
    return assemble_output(res)


# revision 12
# speedup vs baseline: 3.3021x; 1.0516x over previous
"""Trainium2 Bass kernel for nn_Encoder_6 (conv+GN+InterpLnr x3 -> biLSTM).

Self-contained: host-side prep (sharding, interp gather tables, weight
repacking) + Bass/Tile device kernel + output gather.

Data-parallel over 8 NeuronCores: 64 samples per core.

Device dataflow per core (all samples resident on-chip after one load):
  - activations live in [channel(partition), sample, time] layout
  - conv1d = 10-11 accumulating matmuls per sample-pair (taps x cin-chunks),
    f32r (tf32-like) dtype, PSUM [128, 2x192]
  - GroupNorm stats fused into PSUM evacuation (ACT copy+accum -> sums,
    DVE square+accum -> sumsq), group reduce + expand via tiny matmuls
  - normalize+ReLU = single ACT op (per-partition scale/bias)
  - InterpLnr = gpsimd ap_gather along time + 3 DVE ops (w1*g1 + w2*g2)
  - biLSTM: gate preacts via matmuls straight into PSUM blocks; recurrence
    reads gate slices from PSUM (ACT sigmoid/tanh, DVE cell update)
"""
import sys
from contextlib import ExitStack

sys.path.insert(0, "/opt/trn_rl_repo")

import numpy as np
import ml_dtypes

B = 512
N_CORES = 8
S = B // N_CORES          # samples per core
DIM_PIT = 257
C = 256                   # conv channels
T = 192                   # padded time
TH = 196                  # time with halo (2 each side)
GRP = 16                  # channels per group
DIM_NECK = 32
FREQ = 8
NT_OUT = 24               # output timesteps per direction
MIN_LEN_SEG = 19
MAX_NUM_SEG = 7
W64 = 64                  # 2*MAX_LEN_SEG
EPS = 1e-5
SG = 32                   # samples per stats group (2 groups per core)
NPAIR = 16                # sample pairs per stats group
LBLK = 4                  # LSTM timesteps per PSUM block
NBLK = T // LBLK

_cache = {}


# ---------------------------------------------------------------- host prep

def _interp_tables(scales_u, len_seg_raw, n):
    """Gather idx/w1/w2 per sample for one interp layer (numpy, exact)."""
    scales = scales_u.astype(np.float32) + np.float32(0.5)
    j = np.arange(W64, dtype=np.float32)
    idx_scaled = j[None, :] / scales[:, None]
    idx_fl = np.floor(idx_scaled)
    lam = idx_scaled - idx_fl
    len_seg = (len_seg_raw + MIN_LEN_SEG).astype(np.float32)[:, None]
    idx_mask = idx_fl < (len_seg - 1.0)
    ls = (len_seg_raw + MIN_LEN_SEG).reshape(n, MAX_NUM_SEG)
    offset = np.cumsum(ls, axis=-1)
    offset = np.pad(offset[:, :-1], ((0, 0), (1, 0))).reshape(-1, 1)
    idx_org = idx_fl + offset.astype(np.float32)
    mask = (idx_mask & (idx_org < (T - 1))).reshape(n, MAX_NUM_SEG * W64)
    idx_b = np.clip(idx_org.reshape(n, -1).astype(np.int32), 0, T - 2)
    lam_b = lam.reshape(n, -1)
    idx = np.zeros((n, T), np.int32)
    w1 = np.zeros((n, T), np.float32)
    w2 = np.zeros((n, T), np.float32)
    for b in range(n):
        js = np.nonzero(mask[b])[0][:T]
        k = len(js)
        idx[b, :k] = idx_b[b, js]
        w1[b, :k] = 1.0 - lam_b[b, js]
        w2[b, :k] = lam_b[b, js]
    return idx, w1, w2


def _wrap_idx(idx_pairs):
    """[n, NI] int -> ap_gather wrapped layout [n, 128, NI//16] int16."""
    n, NI = idx_pairs.shape
    wrapped = idx_pairs.reshape(n, NI // 16, 16).transpose(0, 2, 1)
    out = np.tile(wrapped[:, None, :, :], (1, 8, 1, 1)).reshape(n, 128, NI // 16)
    return np.ascontiguousarray(out.astype(np.int16))


def _prep_host(inputs):
    """Build per-core input dicts. Returns list of 8 dicts."""
    x = np.asarray(inputs["x"], np.float32)
    scales = np.asarray(inputs["scales"], np.float32)
    lsr = np.asarray(inputs["len_seg_raw"], np.int32)

    # conv weights as lhsT tiles [l, chunk, tap, half, cin128, cout128]
    wconv = np.zeros((3, 2, 5, 2, 128, 128), np.float32)
    for l in range(3):
        w = np.asarray(inputs[f"conv{l}_w"], np.float32)  # [256, cin, 5]
        for cc in range(2):
            for k in range(5):
                for h in range(2):
                    wconv[l, cc, k, h] = w[h * 128:(h + 1) * 128,
                                           cc * 128:(cc + 1) * 128, k].T
    wconv = np.ascontiguousarray(wconv.astype(np.float16))
    # conv0 channel 256 as [5, 256] lhsT (k=tap)
    w0 = np.asarray(inputs["conv0_w"], np.float32)
    wc0e = np.ascontiguousarray(w0[:, 256, :].T.astype(np.float16))  # [5, 256]

    conv_bias = [np.asarray(inputs[f"conv{l}_b"], np.float32) for l in range(3)]
    assert all(np.abs(b).max() == 0.0 for b in conv_bias), \
        "nonzero conv bias not implemented in device kernel"

    gamma_t = np.stack([np.asarray(inputs[f"gn{l}_g"], np.float32).reshape(2, 128)
                        for l in range(3)])          # [3, 2, 128]
    beta_t = np.stack([np.asarray(inputs[f"gn{l}_b"], np.float32).reshape(2, 128)
                       for l in range(3)])
    gamma_t = np.ascontiguousarray(gamma_t.transpose(2, 0, 1).reshape(128, 6))
    beta_t = np.ascontiguousarray(beta_t.transpose(2, 0, 1).reshape(128, 6))

    gind = np.zeros((128, 8), np.float32)
    for c in range(128):
        gind[c, c // 16] = 1.0
    gexp = np.ascontiguousarray(gind.T)               # [8, 128]

    # interp tables, all samples
    idx_all, w1_all, w2_all = [], [], []
    for l in range(3):
        idx, w1, w2 = _interp_tables(scales[l], lsr[l], B)
        idx_all.append(idx)
        w1_all.append(w1)
        w2_all.append(w2)

    # LSTM weights: gate reorder i,f,o,g; lhsT layouts
    def reord(a):  # [128, ...] gate-major
        i_, f_, g_, o_ = np.split(a, 4, axis=0)
        return np.concatenate([i_, f_, o_, g_], axis=0)

    # sample-partition LSTM layouts:
    #  wihT [cc, cin128, (d,gate)]  rhs of xw matmuls (bf16)
    #  whhT [d, h, gate]            rhs of recurrence matmuls (bf16)
    #  lbias4 [d, 4*128]            bias row tiled 4x for rank-1 (f32)
    wihT = np.zeros((2, 128, 256), np.float32)
    whhT = np.zeros((2, 32, 128), np.float32)
    lbias4 = np.zeros((2, 4, 128), np.float32)
    for d, nm in enumerate(["f", "b"]):
        wi = reord(np.asarray(inputs[f"w_ih_{nm}"], np.float32))   # [128, 256]
        wh = reord(np.asarray(inputs[f"w_hh_{nm}"], np.float32))   # [128, 32]
        bb = reord((np.asarray(inputs[f"b_ih_{nm}"], np.float32)
                    + np.asarray(inputs[f"b_hh_{nm}"], np.float32))[:, None])[:, 0]
        for cc in range(2):
            wihT[cc, :, d * 128:(d + 1) * 128] = wi[:, cc * 128:(cc + 1) * 128].T
        whhT[d] = wh.T
        lbias4[d, :, :] = bb[None, :]
    wihT = np.ascontiguousarray(wihT.astype(np.float16))
    whhT = np.ascontiguousarray(whhT.astype(np.float16))
    lbias4 = np.ascontiguousarray(lbias4.reshape(2, 512))

    in_maps = []
    for core in range(N_CORES):
        s0 = core * S
        xs = x[s0:s0 + S]                              # [S, 257, 192]
        xt = xs.transpose(1, 0, 2)                     # [257, S, 192]
        xa = np.zeros((128, S, TH), np.float32)
        xb = np.zeros((128, S, TH), np.float32)
        xa[:, :, 2:194] = xt[:128]
        xb[:, :, 2:194] = xt[128:256]
        xc = np.zeros((5, S, T), np.float32)
        x256 = xt[256]                                 # [S, 192]
        for k in range(5):
            sh = k - 2
            lo, hi = max(0, -sh), min(T, T - sh)
            xc[k, :, lo:hi] = x256[:, lo + sh:hi + sh]

        # banded interp matrices S[t_in, t_out] per (layer, sample), fp16
        wS = np.zeros((3, S, T, T), np.float16)
        bi = np.arange(S)[:, None]
        pj = np.arange(T)[None, :]
        for l in range(3):
            idx = idx_all[l][s0:s0 + S]
            Sm = np.zeros((S, T, T), np.float32)
            Sm[bi, idx, pj] = w1_all[l][s0:s0 + S]
            Sm[bi, idx + 1, pj] += w2_all[l][s0:s0 + S]
            wS[l] = Sm.astype(np.float16)

        in_maps.append({
            "xa": np.ascontiguousarray(xa.astype(np.float16)),
            "xb": np.ascontiguousarray(xb.astype(np.float16)),
            "xc": np.ascontiguousarray(xc.astype(np.float16)),
            "wconv": wconv,
            "wc0e": wc0e,
            "gamma_t": gamma_t,
            "beta_t": beta_t,
            "gind": gind,
            "gexp": gexp,
            "wS": np.ascontiguousarray(wS),
            "id128": np.eye(128, dtype=np.float16),
            "wihT": wihT,
            "whhT": whhT,
            "lbias4": lbias4,
        })
    return in_maps


# ------------------------------------------------------------- device build

def _build(probe_layer=-1):
    """Build the Bacc module. probe_layer >= 0 adds a probe output of XBUF
    after that layer's interp (for debugging)."""
    import concourse.bass as bass
    import concourse.tile as tile
    from concourse import bacc, mybir
    from concourse.masks import make_identity

    f32 = mybir.dt.float32
    f32r = mybir.dt.float32r
    bf16 = mybir.dt.bfloat16
    fp16 = mybir.dt.float16
    i16 = mybir.dt.int16
    AF = mybir.ActivationFunctionType
    OP = mybir.AluOpType

    nc = bacc.Bacc("TRN2", target_bir_lowering=False, debug=False,
                   enable_asserts=False, num_devices=N_CORES)

    # DRAM tensors
    d_xa = nc.dram_tensor("xa", [128, S, TH], fp16, kind="ExternalInput")
    d_xb = nc.dram_tensor("xb", [128, S, TH], fp16, kind="ExternalInput")
    d_xc = nc.dram_tensor("xc", [5, S, T], fp16, kind="ExternalInput")
    d_wconv = nc.dram_tensor("wconv", [3, 2, 5, 2, 128, 128], fp16,
                             kind="ExternalInput")
    d_wc0e = nc.dram_tensor("wc0e", [5, 256], fp16, kind="ExternalInput")
    d_gamma = nc.dram_tensor("gamma_t", [128, 6], f32, kind="ExternalInput")
    d_beta = nc.dram_tensor("beta_t", [128, 6], f32, kind="ExternalInput")
    d_gind = nc.dram_tensor("gind", [128, 8], f32, kind="ExternalInput")
    d_gexp = nc.dram_tensor("gexp", [8, 128], f32, kind="ExternalInput")
    d_wS = nc.dram_tensor("wS", [3, S, T, T], fp16, kind="ExternalInput")
    d_id128 = nc.dram_tensor("id128", [128, 128], fp16, kind="ExternalInput")
    d_wihT = nc.dram_tensor("wihT", [2, 128, 256], fp16, kind="ExternalInput")
    d_whhT = nc.dram_tensor("whhT", [2, 32, 128], fp16, kind="ExternalInput")
    d_lbias4 = nc.dram_tensor("lbias4", [2, 512], f32, kind="ExternalInput")
    d_out = nc.dram_tensor("out", [S, NT_OUT, 64], f32, kind="ExternalOutput")
    d_probe = None
    if probe_layer >= 0:
        d_probe = nc.dram_tensor("probe", [2, 128, S, TH], f32r,
                                 kind="ExternalOutput")

    es = ExitStack()
    with tile.TileContext(nc) as tc, es:
        consts = es.enter_context(tc.tile_pool(name="consts", bufs=1))
        xbufs = es.enter_context(tc.tile_pool(name="xbufs", bufs=1))

        # ---- constants
        t_xc = consts.tile([5, S, T], fp16)
        nc.sync.dma_start(out=t_xc[:], in_=d_xc[:, :, :])
        t_wc0e = consts.tile([5, 256], fp16)
        nc.sync.dma_start(out=t_wc0e[:], in_=d_wc0e[:, :])
        t_gamma = consts.tile([128, 6], f32)
        nc.sync.dma_start(out=t_gamma[:], in_=d_gamma[:, :])
        t_beta = consts.tile([128, 6], f32)
        nc.sync.dma_start(out=t_beta[:], in_=d_beta[:, :])
        t_gind = consts.tile([128, 8], f32)
        nc.sync.dma_start(out=t_gind[:], in_=d_gind[:, :])
        t_gexp = consts.tile([8, 128], f32)
        nc.sync.dma_start(out=t_gexp[:], in_=d_gexp[:, :])
        t_eps = consts.tile([8, 1], f32)
        nc.vector.memset(t_eps[:], EPS)
        t_ones = consts.tile([1, 512], f32)
        nc.vector.memset(t_ones[:], 1.0)
        t_wihT = consts.tile([128, 2, 256], fp16)
        nc.sync.dma_start(
            out=t_wihT[:],
            in_=bass.AP(tensor=d_wihT, offset=0,
                        ap=[[256, 128], [128 * 256, 2], [1, 256]]))
        t_whhT = consts.tile([32, 2, 128], fp16)
        nc.sync.dma_start(
            out=t_whhT[:],
            in_=bass.AP(tensor=d_whhT, offset=0,
                        ap=[[128, 32], [32 * 128, 2], [1, 128]]))
        t_lb4 = consts.tile([1, 2, 512], f32)
        nc.sync.dma_start(out=t_lb4[:], in_=d_lbias4[None, :, :])
        t_id64 = consts.tile([64, 64], f32)
        make_identity(nc, t_id64[:])
        t_id128 = consts.tile([128, 128], fp16)
        nc.sync.dma_start(out=t_id128[:], in_=d_id128[:, :])

        # ---- input activations (xbuf reused as interp output every layer)
        t_xa = xbufs.tile([128, S, TH], fp16)
        t_xb = xbufs.tile([128, S, TH], fp16)
        nc.sync.dma_start(out=t_xa[:], in_=d_xa[:, :, :])
        nc.sync.dma_start(out=t_xb[:], in_=d_xb[:, :, :])
        xbuf = [t_xa, t_xb]

        def mm(out, lhsT, rhs, start, stop, dt=None, **kw):
            if dt is not None:
                lhsT = lhsT.bitcast(dt)
                rhs = rhs.bitcast(dt)
            nc.tensor.matmul(out=out, lhsT=lhsT, rhs=rhs, start=start,
                             stop=stop, **kw)

        # ================= conv + GN + interp layers =================
        with ExitStack() as ces:
            wpool = ces.enter_context(tc.tile_pool(name="wpool", bufs=1))
            hraw_p = ces.enter_context(tc.tile_pool(name="hraw", bufs=1))
            stats_p = ces.enter_context(tc.tile_pool(name="stats", bufs=2))
            small_p = ces.enter_context(tc.tile_pool(name="small", bufs=2))
            y_p = ces.enter_context(tc.tile_pool(name="ybuf", bufs=3))
            scr_p = ces.enter_context(tc.tile_pool(name="scr", bufs=3))
            sm_p = ces.enter_context(tc.tile_pool(name="smat", bufs=2))
            yt_p = ces.enter_context(tc.tile_pool(name="ytp", bufs=3))
            cpsum = ces.enter_context(
                tc.tile_pool(name="cpsum", bufs=2, space="PSUM"))
            stps = ces.enter_context(
                tc.tile_pool(name="stps", bufs=2, space="PSUM"))
            tpsum = ces.enter_context(
                tc.tile_pool(name="tpsum", bufs=2, space="PSUM"))
            sops = ces.enter_context(
                tc.tile_pool(name="sops", bufs=2, space="PSUM"))

            for l in range(3):
                t_wc = wpool.tile([128, 20, 128], fp16, tag="wconv")
                nc.sync.dma_start(
                    out=t_wc[:],
                    in_=bass.AP(tensor=d_wconv, offset=l * 20 * 128 * 128,
                                ap=[[128, 128], [128 * 128, 20], [1, 128]]))

                for grp in range(2):
                    sums = [stats_p.tile([128, SG], f32, tag=f"sums{h}", name=f"sums{h}")
                            for h in range(2)]
                    qs = [stats_p.tile([128, SG], f32, tag=f"qs{h}", name=f"qs{h}")
                          for h in range(2)]
                    hraw = [hraw_p.tile([128, SG, T], fp16, tag=f"hraw{h}", name=f"hraw{h}")
                            for h in range(2)]

                    # ---- phase 1: conv + fused stats
                    for pp in range(NPAIR):
                        pr = grp * NPAIR + pp
                        for h in range(2):
                            ps = cpsum.tile([128, 2, T], f32, tag="cps")
                            ops = []
                            for cc in range(2):
                                for k in range(5):
                                    ops.append((
                                        t_wc[:, (cc * 5 + k) * 2 + h, :],
                                        xbuf[cc][:, 2 * pr:2 * pr + 2,
                                                 k:k + T], None))
                            if l == 0:
                                ops.append((
                                    t_wc0e[:, h * 128:(h + 1) * 128],
                                    t_xc[:, 2 * pr:2 * pr + 2, :], None))
                            for j, (lh, rh, dt) in enumerate(ops):
                                mm(ps[:], lh, rh, j == 0, j == len(ops) - 1,
                                   dt=dt)
                            for i in range(2):
                                sl = pp * 2 + i
                                nc.scalar.activation(
                                    out=hraw[h][:, sl, :], in_=ps[:, i, :],
                                    func=AF.Identity,
                                    accum_out=sums[h][:, sl:sl + 1])
                                scr = scr_p.tile([128, T], fp16, tag="sq")
                                nc.vector.scalar_tensor_tensor(
                                    out=scr[:], in0=ps[:, i, :], scalar=1.0,
                                    in1=hraw[h][:, sl, :], op0=OP.mult,
                                    op1=OP.mult,
                                    accum_out=qs[h][:, sl:sl + 1])

                    # ---- phase 2: group stats -> A, B per half
                    AB = []
                    for h in range(2):
                        g1 = stps.tile([8, SG], f32, tag="gg")
                        mm(g1[:], t_gind[:], sums[h][:], True, True)
                        g2 = stps.tile([8, SG], f32, tag="gg")
                        mm(g2[:], t_gind[:], qs[h][:], True, True)
                        mean = small_p.tile([8, SG], f32, tag="mean")
                        nc.vector.tensor_scalar_mul(mean[:], g1[:],
                                                    1.0 / (GRP * T))
                        msq = small_p.tile([8, SG], f32, tag="msq")
                        nc.vector.tensor_tensor(out=msq[:], in0=mean[:],
                                                in1=mean[:], op=OP.mult)
                        var = small_p.tile([8, SG], f32, tag="var")
                        nc.vector.scalar_tensor_tensor(
                            out=var[:], in0=g2[:], scalar=1.0 / (GRP * T),
                            in1=msq[:], op0=OP.mult, op1=OP.subtract)
                        sd = small_p.tile([8, SG], f32, tag="sd")
                        nc.scalar.activation(out=sd[:], in_=var[:],
                                             func=AF.Sqrt,
                                             bias=t_eps[:, :1], scale=1.0)
                        rstd = small_p.tile([8, SG], f32, tag="rstd")
                        nc.vector.reciprocal(rstd[:], sd[:])
                        rp = stps.tile([128, SG], f32, tag="gg")
                        mm(rp[:], t_gexp[:], rstd[:], True, True)
                        mp = stps.tile([128, SG], f32, tag="gg")
                        mm(mp[:], t_gexp[:], mean[:], True, True)
                        At = small_p.tile([128, SG], f32, tag="A")
                        nc.vector.tensor_scalar_mul(
                            At[:], rp[:],
                            t_gamma[:, l * 2 + h:l * 2 + h + 1])
                        tmp = small_p.tile([128, SG], f32, tag="tmp")
                        nc.vector.tensor_tensor(out=tmp[:], in0=mp[:],
                                                in1=At[:], op=OP.mult)
                        Bt = small_p.tile([128, SG], f32, tag="B")
                        nc.vector.tensor_scalar(
                            out=Bt[:], in0=tmp[:], scalar1=-1.0,
                            scalar2=t_beta[:, l * 2 + h:l * 2 + h + 1],
                            op0=OP.mult, op1=OP.add)
                        AB.append((At, Bt))

                    # ---- phase 3: normalize+relu, transpose, interp matmul
                    for pp in range(NPAIR):
                        pr = grp * NPAIR + pp
                        s128 = sm_p.tile([128, 2, T], fp16, tag="s128")
                        nc.gpsimd.dma_start(
                            out=s128[:],
                            in_=d_wS[l, 2 * pr:2 * pr + 2, 0:128, :].rearrange(
                                "s t w -> t s w"))
                        s64 = sm_p.tile([64, 2, T], fp16, tag="s64")
                        nc.gpsimd.dma_start(
                            out=s64[:],
                            in_=d_wS[l, 2 * pr:2 * pr + 2, 128:192, :].rearrange(
                                "s t w -> t s w"))
                        for i in range(2):
                            sl = pp * 2 + i
                            sg_ = 2 * pr + i
                            yt128 = yt_p.tile([128, 256], fp16, tag="yt128")
                            yt64 = yt_p.tile([64, 256], fp16, tag="yt64")
                            for h in range(2):
                                At, Bt = AB[h]
                                yb = y_p.tile([128, T], fp16, tag="yb")
                                nc.scalar.activation(
                                    out=yb[:], in_=hraw[h][:, sl, :],
                                    func=AF.Relu, bias=Bt[:, sl:sl + 1],
                                    scale=At[:, sl:sl + 1])
                                ptp = tpsum.tile([128, 2, 128], fp16,
                                                 tag="tp", name="ptp")
                                nc.tensor.transpose(
                                    out=ptp[:, 0, :], in_=yb[:, 0:128],
                                    identity=t_id128[:])
                                nc.tensor.transpose(
                                    out=ptp[0:64, 1, :], in_=yb[:, 128:192],
                                    identity=t_id128[:])
                                nc.scalar.activation(
                                    out=yt128[:, h * 128:(h + 1) * 128],
                                    in_=ptp[:, 0, :], func=AF.Identity)
                                nc.vector.tensor_copy(
                                    out=yt64[:, h * 128:(h + 1) * 128],
                                    in_=ptp[0:64, 1, :])
                            sout = sops.tile([128, 2, T], f32, tag="so",
                                             name="sout")
                            for ch in range(2):
                                mm(sout[:, ch, :],
                                   yt128[:, ch * 128:(ch + 1) * 128],
                                   s128[:, i, :], True, False)
                                mm(sout[:, ch, :],
                                   yt64[:, ch * 128:(ch + 1) * 128],
                                   s64[:, i, :], False, True)
                            nc.vector.tensor_copy(
                                out=xbuf[0][:, sg_, 2:194], in_=sout[:, 0, :])
                            nc.vector.tensor_copy(
                                out=xbuf[1][:, sg_, 2:194], in_=sout[:, 1, :])

                if probe_layer == l:
                    for h in range(2):
                        nc.sync.dma_start(out=d_probe[h, :, :, :],
                                          in_=xbuf[h][:, :, :])

        # ======================= biLSTM =======================
        # sample-partition layout: [sample(64 part), gate-unit(col)].
        # h3 copied to bf16 for cheap xw matmuls.
        lsb = es.enter_context(tc.tile_pool(name="lstm_sbuf", bufs=1))
        t_SIG = lsb.tile([64, 256], f32)
        t_C = lsb.tile([64, 64], f32)
        t_U = lsb.tile([64, 64], f32)
        t_FC = lsb.tile([64, 64], f32)
        t_TC = lsb.tile([64, 64], f32)
        t_Hs = lsb.tile([64, 64], f32)
        t_HT = [lsb.tile([32, 64], fp16, name=f"ht{d}") for d in range(2)]
        t_OUT = lsb.tile([S, NT_OUT, 64], f32)
        nc.vector.memset(t_C[:], 0.0)
        nc.vector.memset(t_Hs[:], 0.0)
        for d in range(2):
            nc.vector.memset(t_HT[d][:], 0.0)

        with tc.tile_pool(name="lpsum", bufs=2, space="PSUM") as lpsum, \
             tc.tile_pool(name="tpsum", bufs=2, space="PSUM") as tpsum:

            def xw_block(blk, d):
                """gate preacts, 4 timesteps of dir d -> one PSUM bank."""
                ps = lpsum.tile([64, LBLK, 128], f32, tag=f"xw{d}",
                                name=f"xw{d}")
                for j in range(LBLK):
                    t = blk * LBLK + j if d == 0 else T - 1 - blk * LBLK - j
                    for cc in range(2):
                        mm(ps[:, j, :], xbuf[cc][:, :, 2 + t],
                           t_wihT[:, cc, d * 128:(d + 1) * 128],
                           j == 0 and cc == 0, False)
                mm(ps[:], t_ones[:, 0:64], t_lb4[:, d, :], False, True)
                return ps

            xwp = [[xw_block(0, d), None] for d in range(2)]

            for g in range(T):
                blk, j = g // LBLK, g % LBLK
                for d in range(2):
                    t = g if d == 0 else T - 1 - g
                    ps = xwp[d][blk % 2]
                    slc = ps[:, j, :]
                    dcol = d * 128
                    # gates += h_{t-1} @ whh
                    mm(slc, t_HT[d][:], t_whhT[:, d, :], False, True,
                       skip_group_check=True)
                    nc.scalar.activation(
                        out=t_SIG[:, dcol:dcol + 96], in_=slc[:, 0:96],
                        func=AF.Sigmoid)
                    nc.scalar.activation(
                        out=t_SIG[:, dcol + 96:dcol + 128],
                        in_=slc[:, 96:128], func=AF.Tanh)
                # strided-dir slices of t_SIG: [[128, 2], [1, 32]] at offset
                def sgs(off):
                    a = t_SIG[:]
                    return bass.AP(tensor=a.tensor, offset=a.offset + off,
                                   ap=[a.ap[0], [128, 2], [1, 32]])
                nc.vector.tensor_tensor(out=t_U[:], in0=sgs(0), in1=sgs(96),
                                        op=OP.mult)
                nc.vector.tensor_tensor(out=t_FC[:], in0=sgs(32),
                                        in1=t_C[:], op=OP.mult)
                nc.vector.tensor_tensor(out=t_C[:], in0=t_U[:], in1=t_FC[:],
                                        op=OP.add)
                nc.scalar.activation(out=t_TC[:], in_=t_C[:], func=AF.Tanh)
                nc.vector.tensor_tensor(out=t_Hs[:], in0=sgs(64),
                                        in1=t_TC[:], op=OP.mult)
                for d in range(2):
                    t = g if d == 0 else T - 1 - g
                    pht = tpsum.tile([32, 64], f32, tag="pht", name="pht")
                    nc.tensor.transpose(out=pht[:],
                                        in_=t_Hs[:, d * 32:(d + 1) * 32],
                                        identity=t_id64[:])
                    nc.scalar.activation(out=t_HT[d][:], in_=pht[:],
                                         func=AF.Identity)
                    if d == 0 and t % FREQ == FREQ - 1:
                        nc.vector.tensor_copy(
                            out=t_OUT[:, t // FREQ, 0:32],
                            in_=t_Hs[:, 0:32])
                    if d == 1 and t % FREQ == 0:
                        nc.vector.tensor_copy(
                            out=t_OUT[:, t // FREQ, 32:64],
                            in_=t_Hs[:, 32:64])
                if j == 1 and blk + 1 < NBLK:
                    for d in range(2):
                        xwp[d][(blk + 1) % 2] = xw_block(blk + 1, d)

        nc.sync.dma_start(out=d_out[:, :, :], in_=t_OUT[:])

    nc.compile()
    return nc


def _get_nc(probe_layer=-1):
    key = ("nc", probe_layer)
    if key not in _cache:
        _cache[key] = _build(probe_layer)
    return _cache[key]


def run_on_cores(inputs, probe_layer=-1, trace=False):
    """Build (cached), run on 8 cores; returns (results, BassKernelResults)."""
    from concourse.bass_utils import run_bass_kernel_spmd

    nc = _get_nc(probe_layer)
    in_maps = _prep_host(inputs)
    last_exc = None
    for _ in range(3):
        try:
            res = run_bass_kernel_spmd(nc, in_maps,
                                       core_ids=list(range(N_CORES)),
                                       trace=trace)
            return res
        except Exception as e:  # transient NRT errors happen; retry
            last_exc = e
    raise last_exc


def assemble_output(res):
    out = np.zeros((B, NT_OUT, 64), np.float32)
    for core in range(N_CORES):
        s0 = core * S
        out[s0:s0 + S] = res.results[core]["out"]
    return out


def kernel(**inputs):
    res = run_on_cores(inputs)
    return assemble_output(res)


# revision 13
# speedup vs baseline: 3.4498x; 1.0447x over previous
"""Trainium2 Bass kernel for nn_Encoder_6 (conv+GN+InterpLnr x3 -> biLSTM).

Self-contained: host-side prep (sharding, interp gather tables, weight
repacking) + Bass/Tile device kernel + output gather.

Data-parallel over 8 NeuronCores: 64 samples per core.

Device dataflow per core (all samples resident on-chip after one load):
  - activations live in [channel(partition), sample, time] layout
  - conv1d = 10-11 accumulating matmuls per sample-pair (taps x cin-chunks),
    f32r (tf32-like) dtype, PSUM [128, 2x192]
  - GroupNorm stats fused into PSUM evacuation (ACT copy+accum -> sums,
    DVE square+accum -> sumsq), group reduce + expand via tiny matmuls
  - normalize+ReLU = single ACT op (per-partition scale/bias)
  - InterpLnr = gpsimd ap_gather along time + 3 DVE ops (w1*g1 + w2*g2)
  - biLSTM: gate preacts via matmuls straight into PSUM blocks; recurrence
    reads gate slices from PSUM (ACT sigmoid/tanh, DVE cell update)
"""
import sys
from contextlib import ExitStack

sys.path.insert(0, "/opt/trn_rl_repo")

import numpy as np
import ml_dtypes

B = 512
N_CORES = 8
S = B // N_CORES          # samples per core
DIM_PIT = 257
C = 256                   # conv channels
T = 192                   # padded time
TH = 196                  # time with halo (2 each side)
GRP = 16                  # channels per group
DIM_NECK = 32
FREQ = 8
NT_OUT = 24               # output timesteps per direction
MIN_LEN_SEG = 19
MAX_NUM_SEG = 7
W64 = 64                  # 2*MAX_LEN_SEG
EPS = 1e-5
SG = 32                   # samples per stats group (2 groups per core)
NPAIR = 16                # sample pairs per stats group
LBLK = 4                  # LSTM timesteps per PSUM block
NBLK = T // LBLK

_cache = {}


# ---------------------------------------------------------------- host prep

def _interp_tables(scales_u, len_seg_raw, n):
    """Gather idx/w1/w2 per sample for one interp layer (numpy, exact)."""
    scales = scales_u.astype(np.float32) + np.float32(0.5)
    j = np.arange(W64, dtype=np.float32)
    idx_scaled = j[None, :] / scales[:, None]
    idx_fl = np.floor(idx_scaled)
    lam = idx_scaled - idx_fl
    len_seg = (len_seg_raw + MIN_LEN_SEG).astype(np.float32)[:, None]
    idx_mask = idx_fl < (len_seg - 1.0)
    ls = (len_seg_raw + MIN_LEN_SEG).reshape(n, MAX_NUM_SEG)
    offset = np.cumsum(ls, axis=-1)
    offset = np.pad(offset[:, :-1], ((0, 0), (1, 0))).reshape(-1, 1)
    idx_org = idx_fl + offset.astype(np.float32)
    mask = (idx_mask & (idx_org < (T - 1))).reshape(n, MAX_NUM_SEG * W64)
    idx_b = np.clip(idx_org.reshape(n, -1).astype(np.int32), 0, T - 2)
    lam_b = lam.reshape(n, -1)
    idx = np.zeros((n, T), np.int32)
    w1 = np.zeros((n, T), np.float32)
    w2 = np.zeros((n, T), np.float32)
    for b in range(n):
        js = np.nonzero(mask[b])[0][:T]
        k = len(js)
        idx[b, :k] = idx_b[b, js]
        w1[b, :k] = 1.0 - lam_b[b, js]
        w2[b, :k] = lam_b[b, js]
    return idx, w1, w2


def _wrap_idx(idx_pairs):
    """[n, NI] int -> ap_gather wrapped layout [n, 128, NI//16] int16."""
    n, NI = idx_pairs.shape
    wrapped = idx_pairs.reshape(n, NI // 16, 16).transpose(0, 2, 1)
    out = np.tile(wrapped[:, None, :, :], (1, 8, 1, 1)).reshape(n, 128, NI // 16)
    return np.ascontiguousarray(out.astype(np.int16))


def _prep_host(inputs):
    """Build per-core input dicts. Returns list of 8 dicts."""
    x = np.asarray(inputs["x"], np.float32)
    scales = np.asarray(inputs["scales"], np.float32)
    lsr = np.asarray(inputs["len_seg_raw"], np.int32)

    # conv weights as lhsT tiles [l, chunk, tap, half, cin128, cout128]
    wconv = np.zeros((3, 2, 5, 2, 128, 128), np.float32)
    for l in range(3):
        w = np.asarray(inputs[f"conv{l}_w"], np.float32)  # [256, cin, 5]
        for cc in range(2):
            for k in range(5):
                for h in range(2):
                    wconv[l, cc, k, h] = w[h * 128:(h + 1) * 128,
                                           cc * 128:(cc + 1) * 128, k].T
    wconv = np.ascontiguousarray(wconv.astype(np.float16))
    # conv0 channel 256 as [5, 256] lhsT (k=tap)
    w0 = np.asarray(inputs["conv0_w"], np.float32)
    wc0e = np.ascontiguousarray(w0[:, 256, :].T.astype(np.float16))  # [5, 256]

    conv_bias = [np.asarray(inputs[f"conv{l}_b"], np.float32) for l in range(3)]
    assert all(np.abs(b).max() == 0.0 for b in conv_bias), \
        "nonzero conv bias not implemented in device kernel"

    gamma_t = np.stack([np.asarray(inputs[f"gn{l}_g"], np.float32).reshape(2, 128)
                        for l in range(3)])          # [3, 2, 128]
    beta_t = np.stack([np.asarray(inputs[f"gn{l}_b"], np.float32).reshape(2, 128)
                       for l in range(3)])
    gamma_t = np.ascontiguousarray(gamma_t.transpose(2, 0, 1).reshape(128, 6))
    beta_t = np.ascontiguousarray(beta_t.transpose(2, 0, 1).reshape(128, 6))

    gind = np.zeros((128, 8), np.float32)
    for c in range(128):
        gind[c, c // 16] = 1.0
    gexp = np.ascontiguousarray(gind.T)               # [8, 128]

    # interp tables, all samples
    idx_all, w1_all, w2_all = [], [], []
    for l in range(3):
        idx, w1, w2 = _interp_tables(scales[l], lsr[l], B)
        idx_all.append(idx)
        w1_all.append(w1)
        w2_all.append(w2)

    # LSTM weights: gate reorder i,f,o,g; lhsT layouts
    def reord(a):  # [128, ...] gate-major
        i_, f_, g_, o_ = np.split(a, 4, axis=0)
        return np.concatenate([i_, f_, o_, g_], axis=0)

    # sample-partition LSTM layouts:
    #  wihT [cc, cin128, (d,gate)]  rhs of xw matmuls (bf16)
    #  whhT [d, h, gate]            rhs of recurrence matmuls (bf16)
    #  lbias4 [d, 4*128]            bias row tiled 4x for rank-1 (f32)
    wihT = np.zeros((2, 128, 256), np.float32)
    whhT = np.zeros((2, 32, 128), np.float32)
    lbias4 = np.zeros((2, 4, 128), np.float32)
    for d, nm in enumerate(["f", "b"]):
        wi = reord(np.asarray(inputs[f"w_ih_{nm}"], np.float32))   # [128, 256]
        wh = reord(np.asarray(inputs[f"w_hh_{nm}"], np.float32))   # [128, 32]
        bb = reord((np.asarray(inputs[f"b_ih_{nm}"], np.float32)
                    + np.asarray(inputs[f"b_hh_{nm}"], np.float32))[:, None])[:, 0]
        for cc in range(2):
            wihT[cc, :, d * 128:(d + 1) * 128] = wi[:, cc * 128:(cc + 1) * 128].T
        whhT[d] = wh.T
        lbias4[d, :, :] = bb[None, :]
    wihT = np.ascontiguousarray(wihT.astype(np.float16))
    whhT = np.ascontiguousarray(whhT.astype(np.float16))
    lbias4 = np.ascontiguousarray(lbias4.reshape(2, 512))

    in_maps = []
    for core in range(N_CORES):
        s0 = core * S
        xs = x[s0:s0 + S]                              # [S, 257, 192]
        xt = xs.transpose(1, 0, 2)                     # [257, S, 192]
        xa = np.zeros((128, S, TH), np.float32)
        xb = np.zeros((128, S, TH), np.float32)
        xa[:, :, 2:194] = xt[:128]
        xb[:, :, 2:194] = xt[128:256]
        xc = np.zeros((5, S, T), np.float32)
        x256 = xt[256]                                 # [S, 192]
        for k in range(5):
            sh = k - 2
            lo, hi = max(0, -sh), min(T, T - sh)
            xc[k, :, lo:hi] = x256[:, lo + sh:hi + sh]

        # banded interp matrices S[t_in, t_out] per (layer, sample), fp16
        wS = np.zeros((3, S, T, T), np.float16)
        bi = np.arange(S)[:, None]
        pj = np.arange(T)[None, :]
        for l in range(3):
            idx = idx_all[l][s0:s0 + S]
            Sm = np.zeros((S, T, T), np.float32)
            Sm[bi, idx, pj] = w1_all[l][s0:s0 + S]
            Sm[bi, idx + 1, pj] += w2_all[l][s0:s0 + S]
            wS[l] = Sm.astype(np.float16)

        in_maps.append({
            "xa": np.ascontiguousarray(xa.astype(np.float16)),
            "xb": np.ascontiguousarray(xb.astype(np.float16)),
            "xc": np.ascontiguousarray(xc.astype(np.float16)),
            "wconv": wconv,
            "wc0e": wc0e,
            "gamma_t": gamma_t,
            "beta_t": beta_t,
            "gind": gind,
            "gexp": gexp,
            "wS": np.ascontiguousarray(wS),
            "id128": np.eye(128, dtype=np.float16),
            "wihT": wihT,
            "whhT": whhT,
            "lbias4": lbias4,
        })
    return in_maps


# ------------------------------------------------------------- device build

def _build(probe_layer=-1):
    """Build the Bacc module. probe_layer >= 0 adds a probe output of XBUF
    after that layer's interp (for debugging)."""
    import concourse.bass as bass
    import concourse.tile as tile
    from concourse import bacc, mybir
    from concourse.masks import make_identity

    f32 = mybir.dt.float32
    f32r = mybir.dt.float32r
    bf16 = mybir.dt.bfloat16
    fp16 = mybir.dt.float16
    i16 = mybir.dt.int16
    AF = mybir.ActivationFunctionType
    OP = mybir.AluOpType

    nc = bacc.Bacc("TRN2", target_bir_lowering=False, debug=False,
                   enable_asserts=False, num_devices=N_CORES)

    # DRAM tensors
    d_xa = nc.dram_tensor("xa", [128, S, TH], fp16, kind="ExternalInput")
    d_xb = nc.dram_tensor("xb", [128, S, TH], fp16, kind="ExternalInput")
    d_xc = nc.dram_tensor("xc", [5, S, T], fp16, kind="ExternalInput")
    d_wconv = nc.dram_tensor("wconv", [3, 2, 5, 2, 128, 128], fp16,
                             kind="ExternalInput")
    d_wc0e = nc.dram_tensor("wc0e", [5, 256], fp16, kind="ExternalInput")
    d_gamma = nc.dram_tensor("gamma_t", [128, 6], f32, kind="ExternalInput")
    d_beta = nc.dram_tensor("beta_t", [128, 6], f32, kind="ExternalInput")
    d_gind = nc.dram_tensor("gind", [128, 8], f32, kind="ExternalInput")
    d_gexp = nc.dram_tensor("gexp", [8, 128], f32, kind="ExternalInput")
    d_wS = nc.dram_tensor("wS", [3, S, T, T], fp16, kind="ExternalInput")
    d_id128 = nc.dram_tensor("id128", [128, 128], fp16, kind="ExternalInput")
    d_wihT = nc.dram_tensor("wihT", [2, 128, 256], fp16, kind="ExternalInput")
    d_whhT = nc.dram_tensor("whhT", [2, 32, 128], fp16, kind="ExternalInput")
    d_lbias4 = nc.dram_tensor("lbias4", [2, 512], f32, kind="ExternalInput")
    d_out = nc.dram_tensor("out", [S, NT_OUT, 64], f32, kind="ExternalOutput")
    d_probe = None
    if probe_layer >= 0:
        d_probe = nc.dram_tensor("probe", [2, 128, S, TH], f32r,
                                 kind="ExternalOutput")

    es = ExitStack()
    with tile.TileContext(nc) as tc, es:
        consts = es.enter_context(tc.tile_pool(name="consts", bufs=1))
        xbufs = es.enter_context(tc.tile_pool(name="xbufs", bufs=1))

        # ---- constants
        t_xc = consts.tile([5, S, T], fp16)
        nc.sync.dma_start(out=t_xc[:], in_=d_xc[:, :, :])
        t_wc0e = consts.tile([5, 256], fp16)
        nc.sync.dma_start(out=t_wc0e[:], in_=d_wc0e[:, :])
        t_gamma = consts.tile([128, 6], f32)
        nc.sync.dma_start(out=t_gamma[:], in_=d_gamma[:, :])
        t_beta = consts.tile([128, 6], f32)
        nc.sync.dma_start(out=t_beta[:], in_=d_beta[:, :])
        t_gind = consts.tile([128, 8], f32)
        nc.sync.dma_start(out=t_gind[:], in_=d_gind[:, :])
        t_gexp = consts.tile([8, 128], f32)
        nc.sync.dma_start(out=t_gexp[:], in_=d_gexp[:, :])
        t_eps = consts.tile([8, 1], f32)
        nc.vector.memset(t_eps[:], EPS)
        t_ones = consts.tile([1, 512], f32)
        nc.vector.memset(t_ones[:], 1.0)
        t_wihT = consts.tile([128, 2, 256], fp16)
        nc.sync.dma_start(
            out=t_wihT[:],
            in_=bass.AP(tensor=d_wihT, offset=0,
                        ap=[[256, 128], [128 * 256, 2], [1, 256]]))
        t_whhT = consts.tile([32, 2, 128], fp16)
        nc.sync.dma_start(
            out=t_whhT[:],
            in_=bass.AP(tensor=d_whhT, offset=0,
                        ap=[[128, 32], [32 * 128, 2], [1, 128]]))
        t_lb4 = consts.tile([1, 2, 512], f32)
        nc.sync.dma_start(out=t_lb4[:], in_=d_lbias4[None, :, :])
        t_id64 = consts.tile([64, 64], f32)
        make_identity(nc, t_id64[:])
        t_id128 = consts.tile([128, 128], fp16)
        nc.sync.dma_start(out=t_id128[:], in_=d_id128[:, :])

        # ---- input activations (xbuf reused as interp output every layer)
        t_xa = xbufs.tile([128, S, TH], fp16)
        t_xb = xbufs.tile([128, S, TH], fp16)
        nc.sync.dma_start(out=t_xa[:], in_=d_xa[:, :, :])
        nc.sync.dma_start(out=t_xb[:], in_=d_xb[:, :, :])
        xbuf = [t_xa, t_xb]

        def mm(out, lhsT, rhs, start, stop, dt=None, **kw):
            if dt is not None:
                lhsT = lhsT.bitcast(dt)
                rhs = rhs.bitcast(dt)
            nc.tensor.matmul(out=out, lhsT=lhsT, rhs=rhs, start=start,
                             stop=stop, **kw)

        # ================= conv + GN + interp layers =================
        with ExitStack() as ces:
            wpool = ces.enter_context(tc.tile_pool(name="wpool", bufs=1))
            hraw_p = ces.enter_context(tc.tile_pool(name="hraw", bufs=1))
            stats_p = ces.enter_context(tc.tile_pool(name="stats", bufs=2))
            small_p = ces.enter_context(tc.tile_pool(name="small", bufs=2))
            y_p = ces.enter_context(tc.tile_pool(name="ybuf", bufs=3))
            scr_p = ces.enter_context(tc.tile_pool(name="scr", bufs=3))
            sm_p = ces.enter_context(tc.tile_pool(name="smat", bufs=2))
            yt_p = ces.enter_context(tc.tile_pool(name="ytp", bufs=3))
            cpsum = ces.enter_context(
                tc.tile_pool(name="cpsum", bufs=2, space="PSUM"))
            stps = ces.enter_context(
                tc.tile_pool(name="stps", bufs=2, space="PSUM"))
            tpsum = ces.enter_context(
                tc.tile_pool(name="tpsum", bufs=2, space="PSUM"))
            sops = ces.enter_context(
                tc.tile_pool(name="sops", bufs=2, space="PSUM"))

            for l in range(3):
                t_wc = wpool.tile([128, 20, 128], fp16, tag="wconv")
                nc.sync.dma_start(
                    out=t_wc[:],
                    in_=bass.AP(tensor=d_wconv, offset=l * 20 * 128 * 128,
                                ap=[[128, 128], [128 * 128, 20], [1, 128]]))

                for grp in range(2):
                    sums = [stats_p.tile([128, SG], f32, tag=f"sums{h}", name=f"sums{h}")
                            for h in range(2)]
                    qs = [stats_p.tile([128, SG], f32, tag=f"qs{h}", name=f"qs{h}")
                          for h in range(2)]
                    hraw = [hraw_p.tile([128, SG, T], fp16, tag=f"hraw{h}", name=f"hraw{h}")
                            for h in range(2)]

                    # ---- phase 1: conv + fused stats
                    for pp in range(NPAIR):
                        pr = grp * NPAIR + pp
                        for h in range(2):
                            ps = cpsum.tile([128, 2, T], f32, tag="cps")
                            ops = []
                            for cc in range(2):
                                for k in range(5):
                                    ops.append((
                                        t_wc[:, (cc * 5 + k) * 2 + h, :],
                                        xbuf[cc][:, 2 * pr:2 * pr + 2,
                                                 k:k + T], None))
                            if l == 0:
                                ops.append((
                                    t_wc0e[:, h * 128:(h + 1) * 128],
                                    t_xc[:, 2 * pr:2 * pr + 2, :], None))
                            for j, (lh, rh, dt) in enumerate(ops):
                                mm(ps[:], lh, rh, j == 0, j == len(ops) - 1,
                                   dt=dt)
                            for i in range(2):
                                sl = pp * 2 + i
                                nc.scalar.activation(
                                    out=hraw[h][:, sl, :], in_=ps[:, i, :],
                                    func=AF.Identity,
                                    accum_out=sums[h][:, sl:sl + 1])
                                scr = scr_p.tile([128, T], fp16, tag="sq")
                                nc.vector.scalar_tensor_tensor(
                                    out=scr[:], in0=ps[:, i, :], scalar=1.0,
                                    in1=hraw[h][:, sl, :], op0=OP.mult,
                                    op1=OP.mult,
                                    accum_out=qs[h][:, sl:sl + 1])

                    # ---- phase 2: group stats -> A, B per half
                    AB = []
                    for h in range(2):
                        g1 = stps.tile([8, SG], f32, tag="gg")
                        mm(g1[:], t_gind[:], sums[h][:], True, True)
                        g2 = stps.tile([8, SG], f32, tag="gg")
                        mm(g2[:], t_gind[:], qs[h][:], True, True)
                        mean = small_p.tile([8, SG], f32, tag="mean")
                        nc.vector.tensor_scalar_mul(mean[:], g1[:],
                                                    1.0 / (GRP * T))
                        msq = small_p.tile([8, SG], f32, tag="msq")
                        nc.vector.tensor_tensor(out=msq[:], in0=mean[:],
                                                in1=mean[:], op=OP.mult)
                        var = small_p.tile([8, SG], f32, tag="var")
                        nc.vector.scalar_tensor_tensor(
                            out=var[:], in0=g2[:], scalar=1.0 / (GRP * T),
                            in1=msq[:], op0=OP.mult, op1=OP.subtract)
                        sd = small_p.tile([8, SG], f32, tag="sd")
                        nc.scalar.activation(out=sd[:], in_=var[:],
                                             func=AF.Sqrt,
                                             bias=t_eps[:, :1], scale=1.0)
                        rstd = small_p.tile([8, SG], f32, tag="rstd")
                        nc.vector.reciprocal(rstd[:], sd[:])
                        rp = stps.tile([128, SG], f32, tag="gg")
                        mm(rp[:], t_gexp[:], rstd[:], True, True)
                        mp = stps.tile([128, SG], f32, tag="gg")
                        mm(mp[:], t_gexp[:], mean[:], True, True)
                        At = small_p.tile([128, SG], f32, tag="A")
                        nc.vector.tensor_scalar_mul(
                            At[:], rp[:],
                            t_gamma[:, l * 2 + h:l * 2 + h + 1])
                        tmp = small_p.tile([128, SG], f32, tag="tmp")
                        nc.vector.tensor_tensor(out=tmp[:], in0=mp[:],
                                                in1=At[:], op=OP.mult)
                        Bt = small_p.tile([128, SG], f32, tag="B")
                        nc.vector.tensor_scalar(
                            out=Bt[:], in0=tmp[:], scalar1=-1.0,
                            scalar2=t_beta[:, l * 2 + h:l * 2 + h + 1],
                            op0=OP.mult, op1=OP.add)
                        AB.append((At, Bt))

                    # ---- phase 3: normalize+relu, transpose, interp matmul
                    for pp in range(NPAIR):
                        pr = grp * NPAIR + pp
                        s128 = sm_p.tile([128, 2, T], fp16, tag="s128")
                        nc.gpsimd.dma_start(
                            out=s128[:],
                            in_=d_wS[l, 2 * pr:2 * pr + 2, 0:128, :].rearrange(
                                "s t w -> t s w"))
                        s64 = sm_p.tile([64, 2, T], fp16, tag="s64")
                        nc.gpsimd.dma_start(
                            out=s64[:],
                            in_=d_wS[l, 2 * pr:2 * pr + 2, 128:192, :].rearrange(
                                "s t w -> t s w"))
                        for i in range(2):
                            sl = pp * 2 + i
                            sg_ = 2 * pr + i
                            yt128 = yt_p.tile([128, 256], fp16, tag="yt128")
                            yt64 = yt_p.tile([64, 256], fp16, tag="yt64")
                            for h in range(2):
                                At, Bt = AB[h]
                                yb = y_p.tile([128, T], fp16, tag="yb")
                                nc.scalar.activation(
                                    out=yb[:], in_=hraw[h][:, sl, :],
                                    func=AF.Relu, bias=Bt[:, sl:sl + 1],
                                    scale=At[:, sl:sl + 1])
                                ptp = tpsum.tile([128, 2, 128], fp16,
                                                 tag="tp", name="ptp")
                                nc.tensor.transpose(
                                    out=ptp[:, 0, :], in_=yb[:, 0:128],
                                    identity=t_id128[:])
                                nc.tensor.transpose(
                                    out=ptp[0:64, 1, :], in_=yb[:, 128:192],
                                    identity=t_id128[:])
                                nc.scalar.activation(
                                    out=yt128[:, h * 128:(h + 1) * 128],
                                    in_=ptp[:, 0, :], func=AF.Identity)
                                nc.vector.tensor_copy(
                                    out=yt64[:, h * 128:(h + 1) * 128],
                                    in_=ptp[0:64, 1, :])
                            sout = sops.tile([128, 2, T], f32, tag="so",
                                             name="sout")
                            for ch in range(2):
                                mm(sout[:, ch, :],
                                   yt128[:, ch * 128:(ch + 1) * 128],
                                   s128[:, i, :], True, False)
                                mm(sout[:, ch, :],
                                   yt64[:, ch * 128:(ch + 1) * 128],
                                   s64[:, i, :], False, True)
                            nc.vector.tensor_copy(
                                out=xbuf[0][:, sg_, 2:194], in_=sout[:, 0, :])
                            nc.vector.tensor_copy(
                                out=xbuf[1][:, sg_, 2:194], in_=sout[:, 1, :])

                if probe_layer == l:
                    for h in range(2):
                        nc.sync.dma_start(out=d_probe[h, :, :, :],
                                          in_=xbuf[h][:, :, :])

        # ======================= biLSTM =======================
        # sample-partition layout: [sample(64 part), gate-unit(col)].
        # h3 copied to bf16 for cheap xw matmuls.
        lsb = es.enter_context(tc.tile_pool(name="lstm_sbuf", bufs=1))
        # per-dir state tiles; t_SIGd cols: [i 0:32 | f 32:64 | o 64:96 |
        # g~ 96:128 | c 128:160] so (i,f)*(g~,c) is one DVE op and c-add is
        # a free-dim add.
        t_SIGd = [lsb.tile([64, 160], f32, name=f"sigd{d}") for d in range(2)]
        t_P = [lsb.tile([64, 64], f32, name=f"pp{d}") for d in range(2)]
        t_TC = [lsb.tile([64, 32], f32, name=f"tc{d}") for d in range(2)]
        t_Hd = [lsb.tile([64, 32], fp16, name=f"hd{d}") for d in range(2)]
        t_HT = [lsb.tile([32, 64], fp16, name=f"ht{d}") for d in range(2)]
        t_OUTd = [lsb.tile([S, NT_OUT, 32], f32, name=f"outd{d}")
                  for d in range(2)]
        for d in range(2):
            nc.vector.memset(t_SIGd[d][:, 128:160], 0.0)
            nc.vector.memset(t_HT[d][:], 0.0)

        with tc.tile_pool(name="lpsum", bufs=2, space="PSUM") as lpsum, \
             tc.tile_pool(name="ltp", bufs=2, space="PSUM") as ltp:

            def xw_block(blk, d):
                """gate preacts, LBLK timesteps of dir d -> one PSUM bank."""
                ps = lpsum.tile([64, LBLK, 128], f32, tag=f"xw{d}",
                                name=f"xw{d}")
                for j in range(LBLK):
                    t = blk * LBLK + j if d == 0 else T - 1 - blk * LBLK - j
                    for cc in range(2):
                        mm(ps[:, j, :], xbuf[cc][:, :, 2 + t],
                           t_wihT[:, cc, d * 128:(d + 1) * 128],
                           j == 0 and cc == 0, False)
                mm(ps[:], t_ones[:, 0:64], t_lb4[:, d, :], False, True)
                return ps

            xwp = [[xw_block(0, d), None] for d in range(2)]

            for g in range(T):
                blk, j = g // LBLK, g % LBLK
                for d in range(2):
                    t = g if d == 0 else T - 1 - g
                    ps = xwp[d][blk % 2]
                    slc = ps[:, j, :]
                    sd = t_SIGd[d]
                    mm(slc, t_HT[d][:], t_whhT[:, d, :], False, True,
                       skip_group_check=True)
                    nc.scalar.activation(out=sd[:, 0:96], in_=slc[:, 0:96],
                                         func=AF.Sigmoid)
                    nc.scalar.activation(out=sd[:, 96:128],
                                         in_=slc[:, 96:128], func=AF.Tanh)
                    nc.vector.tensor_tensor(out=t_P[d][:], in0=sd[:, 0:64],
                                            in1=sd[:, 96:160], op=OP.mult)
                    nc.vector.tensor_tensor(out=sd[:, 128:160],
                                            in0=t_P[d][:, 0:32],
                                            in1=t_P[d][:, 32:64], op=OP.add)
                    nc.scalar.activation(out=t_TC[d][:], in_=sd[:, 128:160],
                                         func=AF.Tanh)
                    nc.vector.tensor_tensor(out=t_Hd[d][:], in0=sd[:, 64:96],
                                            in1=t_TC[d][:], op=OP.mult)
                    pht = ltp.tile([32, 64], fp16, tag=f"pht{d}",
                                   name=f"pht{d}")
                    nc.tensor.transpose(out=pht[:], in_=t_Hd[d][:],
                                        identity=t_id128[0:64, 0:64])
                    nc.vector.tensor_copy(out=t_HT[d][:], in_=pht[:])
                    if d == 0 and t % FREQ == FREQ - 1:
                        nc.vector.tensor_copy(
                            out=t_OUTd[0][:, t // FREQ, :], in_=t_Hd[0][:])
                    if d == 1 and t % FREQ == 0:
                        nc.vector.tensor_copy(
                            out=t_OUTd[1][:, t // FREQ, :], in_=t_Hd[1][:])
                if j == 1 and blk + 1 < NBLK:
                    for d in range(2):
                        xwp[d][(blk + 1) % 2] = xw_block(blk + 1, d)

        nc.sync.dma_start(out=d_out[:, :, 0:32], in_=t_OUTd[0][:])
        nc.sync.dma_start(out=d_out[:, :, 32:64], in_=t_OUTd[1][:])


    nc.compile()
    return nc


def _get_nc(probe_layer=-1):
    key = ("nc", probe_layer)
    if key not in _cache:
        _cache[key] = _build(probe_layer)
    return _cache[key]


def run_on_cores(inputs, probe_layer=-1, trace=False):
    """Build (cached), run on 8 cores; returns (results, BassKernelResults)."""
    from concourse.bass_utils import run_bass_kernel_spmd

    nc = _get_nc(probe_layer)
    in_maps = _prep_host(inputs)
    last_exc = None
    for _ in range(3):
        try:
            res = run_bass_kernel_spmd(nc, in_maps,
                                       core_ids=list(range(N_CORES)),
                                       trace=trace)
            return res
        except Exception as e:  # transient NRT errors happen; retry
            last_exc = e
    raise last_exc


def assemble_output(res):
    out = np.zeros((B, NT_OUT, 64), np.float32)
    for core in range(N_CORES):
        s0 = core * S
        out[s0:s0 + S] = res.results[core]["out"]
    return out


def kernel(**inputs):
    res = run_on_cores(inputs)
    return assemble_output(res)


# revision 14
# speedup vs baseline: 3.6755x; 1.0654x over previous
"""Trainium2 Bass kernel for nn_Encoder_6 (conv+GN+InterpLnr x3 -> biLSTM).

Self-contained: host-side prep (sharding, interp gather tables, weight
repacking) + Bass/Tile device kernel + output gather.

Data-parallel over 8 NeuronCores: 64 samples per core.

Device dataflow per core (all samples resident on-chip after one load):
  - activations live in [channel(partition), sample, time] layout
  - conv1d = 10-11 accumulating matmuls per sample-pair (taps x cin-chunks),
    f32r (tf32-like) dtype, PSUM [128, 2x192]
  - GroupNorm stats fused into PSUM evacuation (ACT copy+accum -> sums,
    DVE square+accum -> sumsq), group reduce + expand via tiny matmuls
  - normalize+ReLU = single ACT op (per-partition scale/bias)
  - InterpLnr = gpsimd ap_gather along time + 3 DVE ops (w1*g1 + w2*g2)
  - biLSTM: gate preacts via matmuls straight into PSUM blocks; recurrence
    reads gate slices from PSUM (ACT sigmoid/tanh, DVE cell update)
"""
import sys
from contextlib import ExitStack

sys.path.insert(0, "/opt/trn_rl_repo")

import numpy as np
import ml_dtypes

B = 512
N_CORES = 8
S = B // N_CORES          # samples per core
DIM_PIT = 257
C = 256                   # conv channels
T = 192                   # padded time
TH = 196                  # time with halo (2 each side)
GRP = 16                  # channels per group
DIM_NECK = 32
FREQ = 8
NT_OUT = 24               # output timesteps per direction
MIN_LEN_SEG = 19
MAX_NUM_SEG = 7
W64 = 64                  # 2*MAX_LEN_SEG
EPS = 1e-5
SG = 32                   # samples per stats group (2 groups per core)
NPAIR = 16                # sample pairs per stats group
LBLK = 4                  # LSTM timesteps per PSUM block
NBLK = T // LBLK

_cache = {}


# ---------------------------------------------------------------- host prep

def _interp_tables(scales_u, len_seg_raw, n):
    """Gather idx/w1/w2 per sample for one interp layer (numpy, exact)."""
    scales = scales_u.astype(np.float32) + np.float32(0.5)
    j = np.arange(W64, dtype=np.float32)
    idx_scaled = j[None, :] / scales[:, None]
    idx_fl = np.floor(idx_scaled)
    lam = idx_scaled - idx_fl
    len_seg = (len_seg_raw + MIN_LEN_SEG).astype(np.float32)[:, None]
    idx_mask = idx_fl < (len_seg - 1.0)
    ls = (len_seg_raw + MIN_LEN_SEG).reshape(n, MAX_NUM_SEG)
    offset = np.cumsum(ls, axis=-1)
    offset = np.pad(offset[:, :-1], ((0, 0), (1, 0))).reshape(-1, 1)
    idx_org = idx_fl + offset.astype(np.float32)
    mask = (idx_mask & (idx_org < (T - 1))).reshape(n, MAX_NUM_SEG * W64)
    idx_b = np.clip(idx_org.reshape(n, -1).astype(np.int32), 0, T - 2)
    lam_b = lam.reshape(n, -1)
    idx = np.zeros((n, T), np.int32)
    w1 = np.zeros((n, T), np.float32)
    w2 = np.zeros((n, T), np.float32)
    for b in range(n):
        js = np.nonzero(mask[b])[0][:T]
        k = len(js)
        idx[b, :k] = idx_b[b, js]
        w1[b, :k] = 1.0 - lam_b[b, js]
        w2[b, :k] = lam_b[b, js]
    return idx, w1, w2


def _wrap_idx(idx_pairs):
    """[n, NI] int -> ap_gather wrapped layout [n, 128, NI//16] int16."""
    n, NI = idx_pairs.shape
    wrapped = idx_pairs.reshape(n, NI // 16, 16).transpose(0, 2, 1)
    out = np.tile(wrapped[:, None, :, :], (1, 8, 1, 1)).reshape(n, 128, NI // 16)
    return np.ascontiguousarray(out.astype(np.int16))


def _prep_host(inputs):
    """Build per-core input dicts. Returns list of 8 dicts."""
    x = np.asarray(inputs["x"], np.float32)
    scales = np.asarray(inputs["scales"], np.float32)
    lsr = np.asarray(inputs["len_seg_raw"], np.int32)

    # conv weights as lhsT tiles [l, chunk, tap, half, cin128, cout128]
    wconv = np.zeros((3, 2, 5, 2, 128, 128), np.float32)
    for l in range(3):
        w = np.asarray(inputs[f"conv{l}_w"], np.float32)  # [256, cin, 5]
        for cc in range(2):
            for k in range(5):
                for h in range(2):
                    wconv[l, cc, k, h] = w[h * 128:(h + 1) * 128,
                                           cc * 128:(cc + 1) * 128, k].T
    wconv = np.ascontiguousarray(wconv.astype(np.float16))
    # conv0 channel 256 as [5, 256] lhsT (k=tap)
    w0 = np.asarray(inputs["conv0_w"], np.float32)
    wc0e = np.ascontiguousarray(w0[:, 256, :].T.astype(np.float16))  # [5, 256]

    conv_bias = [np.asarray(inputs[f"conv{l}_b"], np.float32) for l in range(3)]
    assert all(np.abs(b).max() == 0.0 for b in conv_bias), \
        "nonzero conv bias not implemented in device kernel"

    gamma_t = np.stack([np.asarray(inputs[f"gn{l}_g"], np.float32).reshape(2, 128)
                        for l in range(3)])          # [3, 2, 128]
    beta_t = np.stack([np.asarray(inputs[f"gn{l}_b"], np.float32).reshape(2, 128)
                       for l in range(3)])
    gamma_t = np.ascontiguousarray(gamma_t.transpose(2, 0, 1).reshape(128, 6))
    beta_t = np.ascontiguousarray(beta_t.transpose(2, 0, 1).reshape(128, 6))

    gind = np.zeros((128, 8), np.float32)
    for c in range(128):
        gind[c, c // 16] = 1.0
    gexp = np.ascontiguousarray(gind.T)               # [8, 128]

    # interp tables, all samples
    idx_all, w1_all, w2_all = [], [], []
    for l in range(3):
        idx, w1, w2 = _interp_tables(scales[l], lsr[l], B)
        idx_all.append(idx)
        w1_all.append(w1)
        w2_all.append(w2)

    # LSTM weights: gate reorder i,f,o,g; lhsT layouts
    def reord(a):  # [128, ...] gate-major
        i_, f_, g_, o_ = np.split(a, 4, axis=0)
        return np.concatenate([i_, f_, o_, g_], axis=0)

    # sample-partition LSTM layouts:
    #  wihT [cc, cin128, (d,gate)]  rhs of xw matmuls (bf16)
    #  whhT [d, h, gate]            rhs of recurrence matmuls (bf16)
    #  lbias4 [d, 4*128]            bias row tiled 4x for rank-1 (f32)
    wihT = np.zeros((2, 128, 256), np.float32)
    whhT = np.zeros((2, 32, 128), np.float32)
    lbias4 = np.zeros((2, 4, 128), np.float32)
    for d, nm in enumerate(["f", "b"]):
        wi = reord(np.asarray(inputs[f"w_ih_{nm}"], np.float32))   # [128, 256]
        wh = reord(np.asarray(inputs[f"w_hh_{nm}"], np.float32))   # [128, 32]
        bb = reord((np.asarray(inputs[f"b_ih_{nm}"], np.float32)
                    + np.asarray(inputs[f"b_hh_{nm}"], np.float32))[:, None])[:, 0]
        for cc in range(2):
            wihT[cc, :, d * 128:(d + 1) * 128] = wi[:, cc * 128:(cc + 1) * 128].T
        whhT[d] = wh.T
        lbias4[d, :, :] = bb[None, :]
    wihT = np.ascontiguousarray(wihT.astype(np.float16))
    whhT = np.ascontiguousarray(whhT.astype(np.float16))
    lbias4 = np.ascontiguousarray(lbias4.reshape(2, 512).astype(np.float16))

    in_maps = []
    for core in range(N_CORES):
        s0 = core * S
        xs = x[s0:s0 + S]                              # [S, 257, 192]
        xt = xs.transpose(1, 0, 2)                     # [257, S, 192]
        xa = np.zeros((128, S, TH), np.float32)
        xb = np.zeros((128, S, TH), np.float32)
        xa[:, :, 2:194] = xt[:128]
        xb[:, :, 2:194] = xt[128:256]
        xc = np.zeros((5, S, T), np.float32)
        x256 = xt[256]                                 # [S, 192]
        for k in range(5):
            sh = k - 2
            lo, hi = max(0, -sh), min(T, T - sh)
            xc[k, :, lo:hi] = x256[:, lo + sh:hi + sh]

        # banded interp matrices S[t_in, t_out] per (layer, sample), fp16
        wS = np.zeros((3, S, T, T), np.float16)
        bi = np.arange(S)[:, None]
        pj = np.arange(T)[None, :]
        for l in range(3):
            idx = idx_all[l][s0:s0 + S]
            Sm = np.zeros((S, T, T), np.float32)
            Sm[bi, idx, pj] = w1_all[l][s0:s0 + S]
            Sm[bi, idx + 1, pj] += w2_all[l][s0:s0 + S]
            wS[l] = Sm.astype(np.float16)

        in_maps.append({
            "xa": np.ascontiguousarray(xa.astype(np.float16)),
            "xb": np.ascontiguousarray(xb.astype(np.float16)),
            "xc": np.ascontiguousarray(xc.astype(np.float16)),
            "wconv": wconv,
            "wc0e": wc0e,
            "gamma_t": gamma_t,
            "beta_t": beta_t,
            "gind": gind,
            "gexp": gexp,
            "wS": np.ascontiguousarray(wS),
            "id128": np.eye(128, dtype=np.float16),
            "wihT": wihT,
            "whhT": whhT,
            "lbias4": lbias4,
        })
    return in_maps


# ------------------------------------------------------------- device build

def _build(probe_layer=-1):
    """Build the Bacc module. probe_layer >= 0 adds a probe output of XBUF
    after that layer's interp (for debugging)."""
    import concourse.bass as bass
    import concourse.tile as tile
    from concourse import bacc, mybir
    from concourse.masks import make_identity

    f32 = mybir.dt.float32
    f32r = mybir.dt.float32r
    bf16 = mybir.dt.bfloat16
    fp16 = mybir.dt.float16
    i16 = mybir.dt.int16
    AF = mybir.ActivationFunctionType
    OP = mybir.AluOpType

    nc = bacc.Bacc("TRN2", target_bir_lowering=False, debug=False,
                   enable_asserts=False, num_devices=N_CORES)

    # DRAM tensors
    d_xa = nc.dram_tensor("xa", [128, S, TH], fp16, kind="ExternalInput")
    d_xb = nc.dram_tensor("xb", [128, S, TH], fp16, kind="ExternalInput")
    d_xc = nc.dram_tensor("xc", [5, S, T], fp16, kind="ExternalInput")
    d_wconv = nc.dram_tensor("wconv", [3, 2, 5, 2, 128, 128], fp16,
                             kind="ExternalInput")
    d_wc0e = nc.dram_tensor("wc0e", [5, 256], fp16, kind="ExternalInput")
    d_gamma = nc.dram_tensor("gamma_t", [128, 6], f32, kind="ExternalInput")
    d_beta = nc.dram_tensor("beta_t", [128, 6], f32, kind="ExternalInput")
    d_gind = nc.dram_tensor("gind", [128, 8], f32, kind="ExternalInput")
    d_gexp = nc.dram_tensor("gexp", [8, 128], f32, kind="ExternalInput")
    d_wS = nc.dram_tensor("wS", [3, S, T, T], fp16, kind="ExternalInput")
    d_id128 = nc.dram_tensor("id128", [128, 128], fp16, kind="ExternalInput")
    d_wihT = nc.dram_tensor("wihT", [2, 128, 256], fp16, kind="ExternalInput")
    d_whhT = nc.dram_tensor("whhT", [2, 32, 128], fp16, kind="ExternalInput")
    d_lbias4 = nc.dram_tensor("lbias4", [2, 512], fp16, kind="ExternalInput")
    d_out = nc.dram_tensor("out", [S, NT_OUT, 64], f32, kind="ExternalOutput")
    d_probe = None
    if probe_layer >= 0:
        d_probe = nc.dram_tensor("probe", [2, 128, S, TH], f32r,
                                 kind="ExternalOutput")

    es = ExitStack()
    with tile.TileContext(nc) as tc, es:
        consts = es.enter_context(tc.tile_pool(name="consts", bufs=1))
        xbufs = es.enter_context(tc.tile_pool(name="xbufs", bufs=1))

        # ---- constants
        t_xc = consts.tile([5, S, T], fp16)
        nc.sync.dma_start(out=t_xc[:], in_=d_xc[:, :, :])
        t_wc0e = consts.tile([5, 256], fp16)
        nc.sync.dma_start(out=t_wc0e[:], in_=d_wc0e[:, :])
        t_gamma = consts.tile([128, 6], f32)
        nc.sync.dma_start(out=t_gamma[:], in_=d_gamma[:, :])
        t_beta = consts.tile([128, 6], f32)
        nc.sync.dma_start(out=t_beta[:], in_=d_beta[:, :])
        t_gind = consts.tile([128, 8], f32)
        nc.sync.dma_start(out=t_gind[:], in_=d_gind[:, :])
        t_gexp = consts.tile([8, 128], f32)
        nc.sync.dma_start(out=t_gexp[:], in_=d_gexp[:, :])
        t_eps = consts.tile([8, 1], f32)
        nc.vector.memset(t_eps[:], EPS)
        t_ones = consts.tile([1, 512], f32)
        nc.vector.memset(t_ones[:], 1.0)
        t_wihT = consts.tile([128, 2, 256], fp16)
        nc.sync.dma_start(
            out=t_wihT[:],
            in_=bass.AP(tensor=d_wihT, offset=0,
                        ap=[[256, 128], [128 * 256, 2], [1, 256]]))
        t_whhT = consts.tile([32, 2, 128], fp16)
        nc.sync.dma_start(
            out=t_whhT[:],
            in_=bass.AP(tensor=d_whhT, offset=0,
                        ap=[[128, 32], [32 * 128, 2], [1, 128]]))
        t_lb4 = consts.tile([1, 2, 512], fp16)
        nc.sync.dma_start(out=t_lb4[:], in_=d_lbias4[None, :, :])
        t_ones16 = consts.tile([1, 64], fp16)
        nc.vector.memset(t_ones16[:], 1.0)
        t_id64 = consts.tile([64, 64], f32)
        make_identity(nc, t_id64[:])
        t_id128 = consts.tile([128, 128], fp16)
        nc.sync.dma_start(out=t_id128[:], in_=d_id128[:, :])

        # ---- input activations (xbuf reused as interp output every layer)
        t_xa = xbufs.tile([128, S, TH], fp16)
        t_xb = xbufs.tile([128, S, TH], fp16)
        nc.sync.dma_start(out=t_xa[:], in_=d_xa[:, :, :])
        nc.sync.dma_start(out=t_xb[:], in_=d_xb[:, :, :])
        xbuf = [t_xa, t_xb]

        def mm(out, lhsT, rhs, start, stop, dt=None, **kw):
            if dt is not None:
                lhsT = lhsT.bitcast(dt)
                rhs = rhs.bitcast(dt)
            nc.tensor.matmul(out=out, lhsT=lhsT, rhs=rhs, start=start,
                             stop=stop, **kw)

        # ================= conv + GN + interp layers =================
        with ExitStack() as ces:
            wpool = ces.enter_context(tc.tile_pool(name="wpool", bufs=1))
            hraw_p = ces.enter_context(tc.tile_pool(name="hraw", bufs=1))
            stats_p = ces.enter_context(tc.tile_pool(name="stats", bufs=2))
            small_p = ces.enter_context(tc.tile_pool(name="small", bufs=2))
            y_p = ces.enter_context(tc.tile_pool(name="ybuf", bufs=3))
            scr_p = ces.enter_context(tc.tile_pool(name="scr", bufs=3))
            sm_p = ces.enter_context(tc.tile_pool(name="smat", bufs=2))
            yt_p = ces.enter_context(tc.tile_pool(name="ytp", bufs=3))
            cpsum = ces.enter_context(
                tc.tile_pool(name="cpsum", bufs=2, space="PSUM"))
            stps = ces.enter_context(
                tc.tile_pool(name="stps", bufs=2, space="PSUM"))
            tpsum = ces.enter_context(
                tc.tile_pool(name="tpsum", bufs=2, space="PSUM"))
            sops = ces.enter_context(
                tc.tile_pool(name="sops", bufs=2, space="PSUM"))

            for l in range(3):
                t_wc = wpool.tile([128, 20, 128], fp16, tag="wconv")
                nc.sync.dma_start(
                    out=t_wc[:],
                    in_=bass.AP(tensor=d_wconv, offset=l * 20 * 128 * 128,
                                ap=[[128, 128], [128 * 128, 20], [1, 128]]))

                for grp in range(2):
                    sums = [stats_p.tile([128, SG], f32, tag=f"sums{h}", name=f"sums{h}")
                            for h in range(2)]
                    qs = [stats_p.tile([128, SG], f32, tag=f"qs{h}", name=f"qs{h}")
                          for h in range(2)]
                    hraw = [hraw_p.tile([128, SG, T], fp16, tag=f"hraw{h}", name=f"hraw{h}")
                            for h in range(2)]

                    # ---- phase 1: conv + fused stats
                    for pp in range(NPAIR):
                        pr = grp * NPAIR + pp
                        for h in range(2):
                            ps = cpsum.tile([128, 2, T], f32, tag="cps")
                            ops = []
                            for cc in range(2):
                                for k in range(5):
                                    ops.append((
                                        t_wc[:, (cc * 5 + k) * 2 + h, :],
                                        xbuf[cc][:, 2 * pr:2 * pr + 2,
                                                 k:k + T], None))
                            if l == 0:
                                ops.append((
                                    t_wc0e[:, h * 128:(h + 1) * 128],
                                    t_xc[:, 2 * pr:2 * pr + 2, :], None))
                            for j, (lh, rh, dt) in enumerate(ops):
                                mm(ps[:], lh, rh, j == 0, j == len(ops) - 1,
                                   dt=dt)
                            for i in range(2):
                                sl = pp * 2 + i
                                nc.scalar.activation(
                                    out=hraw[h][:, sl, :], in_=ps[:, i, :],
                                    func=AF.Identity,
                                    accum_out=sums[h][:, sl:sl + 1])
                                scr = scr_p.tile([128, T], fp16, tag="sq")
                                nc.vector.scalar_tensor_tensor(
                                    out=scr[:], in0=ps[:, i, :], scalar=1.0,
                                    in1=hraw[h][:, sl, :], op0=OP.mult,
                                    op1=OP.mult,
                                    accum_out=qs[h][:, sl:sl + 1])

                    # ---- phase 2: group stats -> A, B per half
                    AB = []
                    for h in range(2):
                        g1 = stps.tile([8, SG], f32, tag="gg")
                        mm(g1[:], t_gind[:], sums[h][:], True, True)
                        g2 = stps.tile([8, SG], f32, tag="gg")
                        mm(g2[:], t_gind[:], qs[h][:], True, True)
                        mean = small_p.tile([8, SG], f32, tag="mean")
                        nc.vector.tensor_scalar_mul(mean[:], g1[:],
                                                    1.0 / (GRP * T))
                        msq = small_p.tile([8, SG], f32, tag="msq")
                        nc.vector.tensor_tensor(out=msq[:], in0=mean[:],
                                                in1=mean[:], op=OP.mult)
                        var = small_p.tile([8, SG], f32, tag="var")
                        nc.vector.scalar_tensor_tensor(
                            out=var[:], in0=g2[:], scalar=1.0 / (GRP * T),
                            in1=msq[:], op0=OP.mult, op1=OP.subtract)
                        sd = small_p.tile([8, SG], f32, tag="sd")
                        nc.scalar.activation(out=sd[:], in_=var[:],
                                             func=AF.Sqrt,
                                             bias=t_eps[:, :1], scale=1.0)
                        rstd = small_p.tile([8, SG], f32, tag="rstd")
                        nc.vector.reciprocal(rstd[:], sd[:])
                        rp = stps.tile([128, SG], f32, tag="gg")
                        mm(rp[:], t_gexp[:], rstd[:], True, True)
                        mp = stps.tile([128, SG], f32, tag="gg")
                        mm(mp[:], t_gexp[:], mean[:], True, True)
                        At = small_p.tile([128, SG], f32, tag="A")
                        nc.vector.tensor_scalar_mul(
                            At[:], rp[:],
                            t_gamma[:, l * 2 + h:l * 2 + h + 1])
                        tmp = small_p.tile([128, SG], f32, tag="tmp")
                        nc.vector.tensor_tensor(out=tmp[:], in0=mp[:],
                                                in1=At[:], op=OP.mult)
                        Bt = small_p.tile([128, SG], f32, tag="B")
                        nc.vector.tensor_scalar(
                            out=Bt[:], in0=tmp[:], scalar1=-1.0,
                            scalar2=t_beta[:, l * 2 + h:l * 2 + h + 1],
                            op0=OP.mult, op1=OP.add)
                        AB.append((At, Bt))

                    # ---- phase 3: normalize+relu, transpose, interp matmul
                    for pp in range(NPAIR):
                        pr = grp * NPAIR + pp
                        s128 = sm_p.tile([128, 2, T], fp16, tag="s128")
                        nc.gpsimd.dma_start(
                            out=s128[:],
                            in_=d_wS[l, 2 * pr:2 * pr + 2, 0:128, :].rearrange(
                                "s t w -> t s w"))
                        s64 = sm_p.tile([64, 2, T], fp16, tag="s64")
                        nc.gpsimd.dma_start(
                            out=s64[:],
                            in_=d_wS[l, 2 * pr:2 * pr + 2, 128:192, :].rearrange(
                                "s t w -> t s w"))
                        for i in range(2):
                            sl = pp * 2 + i
                            sg_ = 2 * pr + i
                            yt128 = yt_p.tile([128, 256], fp16, tag="yt128")
                            yt64 = yt_p.tile([64, 256], fp16, tag="yt64")
                            for h in range(2):
                                At, Bt = AB[h]
                                yb = y_p.tile([128, T], fp16, tag="yb")
                                nc.scalar.activation(
                                    out=yb[:], in_=hraw[h][:, sl, :],
                                    func=AF.Relu, bias=Bt[:, sl:sl + 1],
                                    scale=At[:, sl:sl + 1])
                                ptp = tpsum.tile([128, 2, 128], fp16,
                                                 tag="tp", name="ptp")
                                nc.tensor.transpose(
                                    out=ptp[:, 0, :], in_=yb[:, 0:128],
                                    identity=t_id128[:])
                                nc.tensor.transpose(
                                    out=ptp[0:64, 1, :], in_=yb[:, 128:192],
                                    identity=t_id128[:])
                                nc.scalar.activation(
                                    out=yt128[:, h * 128:(h + 1) * 128],
                                    in_=ptp[:, 0, :], func=AF.Identity)
                                nc.vector.tensor_copy(
                                    out=yt64[:, h * 128:(h + 1) * 128],
                                    in_=ptp[0:64, 1, :])
                            sout = sops.tile([128, 2, T], f32, tag="so",
                                             name="sout")
                            for ch in range(2):
                                mm(sout[:, ch, :],
                                   yt128[:, ch * 128:(ch + 1) * 128],
                                   s128[:, i, :], True, False)
                                mm(sout[:, ch, :],
                                   yt64[:, ch * 128:(ch + 1) * 128],
                                   s64[:, i, :], False, True)
                            nc.vector.tensor_copy(
                                out=xbuf[0][:, sg_, 2:194], in_=sout[:, 0, :])
                            nc.vector.tensor_copy(
                                out=xbuf[1][:, sg_, 2:194], in_=sout[:, 1, :])

                if probe_layer == l:
                    for h in range(2):
                        nc.sync.dma_start(out=d_probe[h, :, :, :],
                                          in_=xbuf[h][:, :, :])

        # ======================= biLSTM =======================
        # sample-partition layout: [sample(64 part), gate-unit(col)].
        # h3 copied to bf16 for cheap xw matmuls.
        lsb = es.enter_context(tc.tile_pool(name="lstm_sbuf", bufs=1))
        # per-dir state tiles; t_SIGd cols: [i 0:32 | f 32:64 | o 64:96 |
        # g~ 96:128 | c 128:160] so (i,f)*(g~,c) is one DVE op and c-add is
        # a free-dim add.
        t_SIGd = [lsb.tile([64, 160], f32, name=f"sigd{d}") for d in range(2)]
        t_P = [lsb.tile([64, 64], f32, name=f"pp{d}") for d in range(2)]
        t_TC = [lsb.tile([64, 32], f32, name=f"tc{d}") for d in range(2)]
        t_Hd = [lsb.tile([64, 32], fp16, name=f"hd{d}") for d in range(2)]
        t_HT = [lsb.tile([32, 64], fp16, name=f"ht{d}") for d in range(2)]
        t_OUTd = [lsb.tile([S, NT_OUT, 32], f32, name=f"outd{d}")
                  for d in range(2)]
        for d in range(2):
            nc.vector.memset(t_SIGd[d][:, 128:160], 0.0)
            nc.vector.memset(t_HT[d][:], 0.0)

        with tc.tile_pool(name="lpsum", bufs=2, space="PSUM") as lpsum, \
             tc.tile_pool(name="ltp", bufs=2, space="PSUM") as ltp:

            def xw_block(blk, d):
                """gate preacts, LBLK timesteps of dir d -> one PSUM bank."""
                ps = lpsum.tile([64, LBLK, 128], f32, tag=f"xw{d}",
                                name=f"xw{d}")
                for j in range(LBLK):
                    t = blk * LBLK + j if d == 0 else T - 1 - blk * LBLK - j
                    for cc in range(2):
                        mm(ps[:, j, :], xbuf[cc][:, :, 2 + t],
                           t_wihT[:, cc, d * 128:(d + 1) * 128],
                           j == 0 and cc == 0, False)
                mm(ps[:], t_ones16[:], t_lb4[:, d, :], False, True)
                return ps

            xwp = [[xw_block(0, d), None] for d in range(2)]

            for g in range(T):
                blk, j = g // LBLK, g % LBLK
                slcs = []
                for d in range(2):
                    ps = xwp[d][blk % 2]
                    slcs.append(ps[:, j, :])
                # level-ordered emission: both dir-chains advance in lockstep
                for d in range(2):
                    mm(slcs[d], t_HT[d][:], t_whhT[:, d, :], False, True,
                       skip_group_check=True)
                for d in range(2):
                    nc.scalar.activation(out=t_SIGd[d][:, 0:96],
                                         in_=slcs[d][:, 0:96],
                                         func=AF.Sigmoid)
                for d in range(2):
                    nc.scalar.activation(out=t_SIGd[d][:, 96:128],
                                         in_=slcs[d][:, 96:128], func=AF.Tanh)
                for d in range(2):
                    nc.vector.tensor_tensor(out=t_P[d][:],
                                            in0=t_SIGd[d][:, 0:64],
                                            in1=t_SIGd[d][:, 96:160],
                                            op=OP.mult)
                for d in range(2):
                    nc.vector.tensor_tensor(out=t_SIGd[d][:, 128:160],
                                            in0=t_P[d][:, 0:32],
                                            in1=t_P[d][:, 32:64], op=OP.add)
                for d in range(2):
                    nc.scalar.activation(out=t_TC[d][:],
                                         in_=t_SIGd[d][:, 128:160],
                                         func=AF.Tanh)
                for d in range(2):
                    nc.vector.tensor_tensor(out=t_Hd[d][:],
                                            in0=t_SIGd[d][:, 64:96],
                                            in1=t_TC[d][:], op=OP.mult)
                phts = []
                for d in range(2):
                    pht = ltp.tile([32, 64], fp16, tag=f"pht{d}",
                                   name=f"pht{d}")
                    nc.tensor.transpose(out=pht[:], in_=t_Hd[d][:],
                                        identity=t_id128[0:64, 0:64])
                    phts.append(pht)
                for d in range(2):
                    nc.vector.tensor_copy(out=t_HT[d][:], in_=phts[d][:])
                t0 = g
                if t0 % FREQ == FREQ - 1:
                    nc.vector.tensor_copy(
                        out=t_OUTd[0][:, t0 // FREQ, :], in_=t_Hd[0][:])
                t1 = T - 1 - g
                if t1 % FREQ == 0:
                    nc.vector.tensor_copy(
                        out=t_OUTd[1][:, t1 // FREQ, :], in_=t_Hd[1][:])
                if j == 1 and blk + 1 < NBLK:
                    for d in range(2):
                        xwp[d][(blk + 1) % 2] = xw_block(blk + 1, d)

        nc.sync.dma_start(out=d_out[:, :, 0:32], in_=t_OUTd[0][:])
        nc.sync.dma_start(out=d_out[:, :, 32:64], in_=t_OUTd[1][:])


    nc.compile()
    return nc


def _get_nc(probe_layer=-1):
    key = ("nc", probe_layer)
    if key not in _cache:
        _cache[key] = _build(probe_layer)
    return _cache[key]


def run_on_cores(inputs, probe_layer=-1, trace=False):
    """Build (cached), run on 8 cores; returns (results, BassKernelResults)."""
    from concourse.bass_utils import run_bass_kernel_spmd

    nc = _get_nc(probe_layer)
    in_maps = _prep_host(inputs)
    last_exc = None
    for _ in range(3):
        try:
            res = run_bass_kernel_spmd(nc, in_maps,
                                       core_ids=list(range(N_CORES)),
                                       trace=trace)
            return res
        except Exception as e:  # transient NRT errors happen; retry
            last_exc = e
    raise last_exc


def assemble_output(res):
    out = np.zeros((B, NT_OUT, 64), np.float32)
    for core in range(N_CORES):
        s0 = core * S
        out[s0:s0 + S] = res.results[core]["out"]
    return out


def kernel(**inputs):
    res = run_on_cores(inputs)
    return assemble_output(res)
